# revision 46
# baseline (speedup 1.0000x reference)
"""Distributed attention kernel for Trainium2 (8 NeuronCores, SPMD).

Problem: B=16 batches of single-query attention over NK=4096 keys,
EMBED=1024, ATTN=256, with a shared kq projection and a v projection.

Math restructuring (exact up to float reassociation):
  - scores = (q@W_kq + b_kq) @ (k@W_kq + b_kq)^T / 16
           = k @ qt + const            where qt = W_kq @ (W_kq^T q + b_kq) / 16
    (the constant offsets every score equally -> softmax invariant, dropped)
  - out = softmax(scores) @ (v@W_v + b_v)
        = (attn @ v) @ W_v + b_v       (attn sums to 1)
This removes the O(NK*E*E) v-projection and O(NK*E*A) k-projection
entirely; the kernel is HBM-bandwidth bound streaming k and v once.

Sharding: data-parallel over batch, 2 batches per core. k, v, W_v are
cast to bf16 on the host (compute dtype of the streaming contractions,
halves HBM traffic); W_kq is additionally passed pre-transposed.

Token layout is p-major ("(p s) d"): partition p holds NSUB consecutive
token rows, so each chunk DMA is one 16KB-contiguous run per partition
(128 descriptors, not 1024). k and v use the same permutation, and
softmax is globally permutation-invariant, so results are unchanged.

s = k.qt is load-balanced across engines per 128-token tile (per chunk
of 8 tiles: 5 via DVE bf16 multiply (2x mode) + ACT activation-accum
row-sum; 3 via DVE fused scalar_tensor_tensor). w = attn@v and the
projections run on TensorE in bf16 (single pass). Softmax uses
unnormalized exp (scores ~ N(0,1), no overflow in fp32) with 1/Z
folded into the w_row evacuation. Batch 1's DMAs are interleaved with
batch 0's v stream so its k chunks are resident when compute frees up.

Raw bass (not Tile): this toolchain's walrus build rejects >1 embedded
sync-wait per compute instruction, which Tile's scheduler emits; raw
bass uses standalone sequencer waits, with explicit semaphore ticks
precomputed in python (the *_seq tables below).

PSUM bank map (PE-W vs DVE/ACT-R hazards serialized via the sem chain;
bank granularity, not address: concurrent PE-write + DVE/ACT-read of
the SAME bank is a fatal HW error even at different addresses):
  bank 0    : qp_row [0:1,256:512], qp_col [:,0:2], Z [0:1,4:5],
              w fold [:,8:16], q fold [:,16:24]
  banks 1-2 : qt row / out row (disjoint lifetimes)
  banks 3-4 : qt broadcast [128, 1024]
  banks 5-6 : w accumulator row
"""

import contextlib

import numpy as np

try:
    import concourse.bass as bass  # noqa: F401
except ImportError:  # fallback if site path isn't preloaded
    import sys

    sys.path.insert(0, "/opt/trn_rl_repo")

B = 16
NCORES = 8
BPC = B // NCORES  # batches per core
NK = 4096
E = 1024
A = 256
NSUB = NK // 128   # 32 token subtiles of 128
CHUNK = 8          # subtiles per DMA chunk (2 MB in bf16)
NCHUNK = NSUB // CHUNK
KBUFS = 4
VBUFS = 3
NACT = 5           # tiles per chunk reduced on ACT (rest fused on DVE)
# scratch slot j is dedicated to tile position j of each chunk, so a
# chunk's multiplies only wait on the PREVIOUS chunk's reduces


def _build_nc():
    import concourse.bass as bass
    from concourse import mybir

    FP = mybir.dt.float32
    BF = mybir.dt.bfloat16
    AL = mybir.AluOpType
    AF = mybir.ActivationFunctionType

    nc = bass.Bass()
    q_d = nc.declare_dram_parameter("q", [BPC, E], BF, isOutput=False)
    k_d = nc.declare_dram_parameter("k", [BPC, NK, E], BF, isOutput=False)
    v_d = nc.declare_dram_parameter("v", [BPC, NK, E], BF, isOutput=False)
    wkq_d = nc.declare_dram_parameter("W_kq", [E, A], BF, isOutput=False)
    wkqT_d = nc.declare_dram_parameter("W_kqT", [A, E], BF, isOutput=False)
    bkq_d = nc.declare_dram_parameter("b_kq", [A], FP, isOutput=False)
    wv_d = nc.declare_dram_parameter("W_v", [E, E], BF, isOutput=False)
    bv_d = nc.declare_dram_parameter("b_v", [E], FP, isOutput=False)
    out_d = nc.declare_dram_parameter("out", [BPC, E], FP, isOutput=True)

    with contextlib.ExitStack() as st:
        def sb(name, shape, dt=FP):
            return st.enter_context(nc.sbuf_tensor(name, shape, dt))

        # ---- SBUF ----
        wkq_sb = sb("wkq_sb", [128, 8, A], BF)
        wkqT_sb = sb("wkqT_sb", [128, 2, E], BF)
        wv_sb = sb("wv_sb", [128, 8, E], BF)
        q_row = sb("q_row", [1, BPC * E], BF)
        qcol_sb = [sb(f"qcol_sb{b}", [128, 8], BF) for b in range(BPC)]
        bkq_row = sb("bkq_row", [1, A])
        bv_row = sb("bv_row", [1, E])
        ones_col = sb("ones_col", [128, 1])        # fp32 (Z rhs)
        ones_bf = sb("ones_bf", [1, 128], BF)      # bf16 (w fold rhs)
        kt = [sb(f"kt{i}", [128, CHUNK, E], BF) for i in range(KBUFS)]
        vt = [sb(f"vt{i}", [128, CHUNK, E], BF) for i in range(VBUFS)]
        scr = [sb(f"scr{i}", [128, E], BF) for i in range(NACT)]
        junk = [sb(f"junk{i}", [128, E], BF) for i in range(CHUNK - NACT)]
        # per-batch smalls
        qpr_sb = [sb(f"qpr_sb{b}", [1, A], BF) for b in range(BPC)]
        qp_sb = [sb(f"qp_sb{b}", [128, 2], BF) for b in range(BPC)]
        qt_sb = [sb(f"qt_sb{b}", [1, E], BF) for b in range(BPC)]
        qtb_sb = [sb(f"qtb_sb{b}", [128, E], BF) for b in range(BPC)]
        smat = [sb(f"smat{b}", [128, NSUB]) for b in range(BPC)]
        pmat = [sb(f"pmat{b}", [128, NSUB], BF) for b in range(BPC)]
        psums = [sb(f"psums{b}", [128, 1]) for b in range(BPC)]
        invz = [sb(f"invz{b}", [1, 1]) for b in range(BPC)]
        w_row = [sb(f"w_row{b}", [1, E], BF) for b in range(BPC)]
        w_col = [sb(f"w_col{b}", [128, 8], BF) for b in range(BPC)]
        o_sb = [sb(f"o_sb{b}", [1, E]) for b in range(BPC)]

        # ---- PSUM (static bank map) ----
        ps_small = st.enter_context(nc.psum_tensor([128, 512], FP))   # bank 0
        ps_a = st.enter_context(nc.psum_tensor([128, 1024], FP))      # banks 1-2
        ps_b = st.enter_context(nc.psum_tensor([128, 1024], FP))      # banks 3-4
        ps_w = st.enter_context(nc.psum_tensor([128, 1024], FP))      # banks 5-6

        # ---- semaphores ----
        sW = st.enter_context(nc.semaphore("sW"))      # wkq+wkqT+q+bkq -> 64
        sWV = st.enter_context(nc.semaphore("sWV"))    # wv -> 16
        sBV = st.enter_context(nc.semaphore("sBV"))    # bv -> 16
        sK = [st.enter_context(nc.semaphore(f"sK{i}")) for i in range(KBUFS)]
        sV = [st.enter_context(nc.semaphore(f"sV{i}")) for i in range(VBUFS)]
        sOUT = st.enter_context(nc.semaphore("sOUT"))
        sPE = st.enter_context(nc.semaphore("sPE"))
        sDVE = st.enter_context(nc.semaphore("sDVE"))
        sACT = st.enter_context(nc.semaphore("sACT"))

        blk = st.enter_context(nc.Block())

        # ---------- event tick registry ----------
        def ticks(seq):
            return {ev: i + 1 for i, ev in enumerate(seq)}

        pe_seq = ["QF0", "QPROW0", "QF1", "QPF0", "QT0", "QTB0",
                  "QPROW1", "QPF1", "QT1", "QTB1"]
        pe_seq += ["Z0"] + [f"W0C{c}" for c in range(NCHUNK)]
        pe_seq += ["Z1", "W1C0", "FOLD0", "W1C1", "PROJ0", "W1C2", "W1C3",
                   "FOLD1", "PROJ1"]
        PE = ticks(pe_seq)

        def mult_ev(b, c, j):
            # DVE inc for tile j of chunk (b, c): mult (j < NACT) or fused stt
            return f"MUL{b}_{c}_{j}"

        def red_ev(b, c, j):
            return f"RED{b}_{c}_{j}"

        dve_seq = ["MS1", "MS2", "QCOL0", "QPRSB0", "QCOL1", "QPSB0",
                   "QTBSB0"]
        dve_seq += [mult_ev(0, 0, j) for j in range(CHUNK)]
        dve_seq += ["QPRSB1"]
        dve_seq += [mult_ev(0, 1, j) for j in range(CHUNK)]
        dve_seq += ["QPSB1"]
        dve_seq += [mult_ev(0, 2, j) for j in range(CHUNK)]
        dve_seq += ["QTBSB1"]
        dve_seq += [mult_ev(0, 3, j) for j in range(CHUNK)]
        dve_seq += [mult_ev(1, 0, j) for j in range(CHUNK)]
        dve_seq += ["INVZ0"]
        dve_seq += [mult_ev(1, 1, j) for j in range(CHUNK)]
        dve_seq += [mult_ev(1, 2, j) for j in range(CHUNK)]
        dve_seq += [mult_ev(1, 3, j) for j in range(CHUNK)]
        dve_seq += ["WCOL0", "OSB0", "INVZ1", "WCOL1", "OSB1"]
        DVE = ticks(dve_seq)

        act_seq = ["QTSB0"]
        act_seq += [red_ev(0, 0, j) for j in range(NACT)]
        act_seq += ["QTSB1"]
        for c in range(1, NCHUNK):
            act_seq += [red_ev(0, c, j) for j in range(NACT)]
        act_seq += ["EXP0"]
        for c in range(NCHUNK):
            act_seq += [red_ev(1, c, j) for j in range(NACT)]
        act_seq += ["EXP1", "WROW0", "WROW1"]
        ACT = ticks(act_seq)

        # ---------- SYNC: all DMAs ----------
        @blk.sync
        def _(sync):
            sync.dma_start(
                out=wkq_sb[:], in_=wkq_d[:].rearrange("(dc p) a -> p dc a", p=128)
            ).then_inc(sW, 16)
            sync.dma_start(
                out=wkqT_sb[:], in_=wkqT_d[:].rearrange("(ac p) d -> p ac d", p=128)
            ).then_inc(sW, 16)
            sync.dma_start(
                out=q_row[:], in_=q_d[:].rearrange("b e -> (b e)")[None, :]
            ).then_inc(sW, 16)
            sync.dma_start(out=bkq_row[:], in_=bkq_d[:][None, :]).then_inc(sW, 16)
            sync.dma_start(out=bv_row[:], in_=bv_d[:][None, :]).then_inc(sBV, 16)

            def kdma(b, c):
                g = b * NCHUNK + c
                if g >= KBUFS:
                    gp = g - KBUFS
                    bp, cp = divmod(gp, NCHUNK)
                    sync.wait_ge(sDVE, DVE[mult_ev(bp, cp, CHUNK - 1)])
                k_b = k_d[:][b].rearrange("(p s) d -> p s d", p=128)
                sync.dma_start(
                    out=kt[g % KBUFS][:],
                    in_=k_b[:, c * CHUNK:(c + 1) * CHUNK, :],
                ).then_inc(sK[g % KBUFS], 16)

            def vdma(b, c):
                g = b * NCHUNK + c
                if g >= VBUFS:
                    gp = g - VBUFS
                    bp, cp = divmod(gp, NCHUNK)
                    sync.wait_ge(sPE, PE[f"W{bp}C{cp}"])
                v_b = v_d[:][b].rearrange("(p s) d -> p s d", p=128)
                sync.dma_start(
                    out=vt[g % VBUFS][:],
                    in_=v_b[:, c * CHUNK:(c + 1) * CHUNK, :],
                ).then_inc(sV[g % VBUFS], 16)

            # k1 interleaved with v0 so it streams during k0/v0 compute;
            # PE-gated issues (v0c3, k1c3) pushed late to avoid head-of-line
            for c in range(NCHUNK):
                kdma(0, c)
            sync.dma_start(
                out=wv_sb[:], in_=wv_d[:].rearrange("(dc p) e -> p dc e", p=128)
            ).then_inc(sWV, 16)
            vdma(0, 0)
            vdma(0, 1)
            vdma(0, 2)
            kdma(1, 0)
            kdma(1, 1)
            kdma(1, 2)
            kdma(1, 3)
            vdma(0, 3)
            for c in range(NCHUNK):
                vdma(1, c)

            for b in range(BPC):
                sync.wait_ge(sDVE, DVE[f"OSB{b}"])
                sync.dma_start(out=out_d[:][b:b + 1, :], in_=o_sb[b][:]).then_inc(
                    sOUT, 16)
            sync.wait_ge(sOUT, BPC * 16)

        # ---------- PE ----------
        @blk.tensor
        def _(tensor):
            tensor.wait_ge(sW, 64)
            tensor.wait_ge(sDVE, DVE["MS2"])  # ones tiles ready
            def qfold(b):
                if b > 0:
                    # bank-0 serialization: latest possible concurrent reader
                    tensor.wait_ge(sDVE, DVE[f"QPRSB{b - 1}"])
                for dc in range(8):
                    mm = tensor.matmul(
                        out=ps_small[:, 16 + dc:17 + dc],
                        lhsT=q_row[0:1, b * E + dc * 128:b * E + (dc + 1) * 128],
                        rhs=ones_bf[0:1, 0:1],
                        start=True, stop=True,
                    )
                mm.then_inc(sPE, 1)                      # QF{b}

            def qprow(b):
                tensor.wait_ge(sDVE, DVE[f"QCOL{b}"])
                if b > 0:
                    # bank-0 safety: prior batch's bank-0 reads done
                    tensor.wait_ge(sDVE, DVE[f"QPSB{b - 1}"])
                for dc in range(8):
                    mm = tensor.matmul(
                        out=ps_small[0:1, 256:256 + A],
                        lhsT=qcol_sb[b][:, dc:dc + 1],
                        rhs=wkq_sb[:, dc, :],
                        start=(dc == 0), stop=(dc == 7),
                    )
                mm.then_inc(sPE, 1)                      # QPROW{b}

            def qpf(b):
                tensor.wait_ge(sDVE, DVE[f"QPRSB{b}"])
                if b == 0:
                    # bank-0 serialization vs QCOL1's read
                    tensor.wait_ge(sDVE, DVE["QCOL1"])
                for c2 in range(2):
                    mm = tensor.matmul(
                        out=ps_small[:, c2:c2 + 1],
                        lhsT=qpr_sb[b][0:1, c2 * 128:(c2 + 1) * 128],
                        rhs=ones_bf[0:1, 0:1],
                        start=True, stop=True,
                    )
                mm.then_inc(sPE, 1)                      # QPF{b}

            def qt_mm(b):
                tensor.wait_ge(sDVE, DVE[f"QPSB{b}"])
                if b > 0:
                    tensor.wait_ge(sACT, ACT[f"QTSB{b - 1}"])
                for ac in range(2):
                    for nh in range(2):
                        mm = tensor.matmul(
                            out=ps_a[0:1, nh * 512:(nh + 1) * 512],
                            lhsT=qp_sb[b][:, ac:ac + 1],
                            rhs=wkqT_sb[:, ac, nh * 512:(nh + 1) * 512],
                            start=(ac == 0), stop=(ac == 1),
                        )
                mm.then_inc(sPE, 1)                      # QT{b}

            def qtb_mm(b):
                tensor.wait_ge(sACT, ACT[f"QTSB{b}"])
                for nh in range(2):
                    mm = tensor.matmul(
                        out=ps_b[:, nh * 512:(nh + 1) * 512],
                        lhsT=ones_bf[:],
                        rhs=qt_sb[b][0:1, nh * 512:(nh + 1) * 512],
                        start=True, stop=True,
                    )
                mm.then_inc(sPE, 1)                      # QTB{b}

            qfold(0)
            qprow(0)
            qfold(1)
            qpf(0)
            qt_mm(0)
            qtb_mm(0)
            qprow(1)
            qpf(1)
            qt_mm(1)
            qtb_mm(1)

            def z_mm(b):
                tensor.wait_ge(sACT, ACT[f"EXP{b}"])
                tensor.matmul(
                    out=ps_small[0:1, 4:5], lhsT=psums[b][:], rhs=ones_col[:],
                    start=True, stop=True,
                ).then_inc(sPE, 1)                       # Z{b}

            def w_chunk(b, c):
                g = b * NCHUNK + c
                tensor.wait_ge(sV[g % VBUFS], (g // VBUFS + 1) * 16)
                if b == 1 and c == 0:
                    # ps_w WAR: batch-0 w_row evacuation must complete
                    tensor.wait_ge(sACT, ACT["WROW0"])
                for j in range(CHUNK):
                    t = c * CHUNK + j
                    for nh in range(2):
                        mm = tensor.matmul(
                            out=ps_w[0:1, nh * 512:(nh + 1) * 512],
                            lhsT=pmat[b][:, t:t + 1],
                            rhs=vt[g % VBUFS][:, j, nh * 512:(nh + 1) * 512],
                            start=(t == 0), stop=(t == NSUB - 1),
                        )
                mm.then_inc(sPE, 1)                      # W{b}C{c}

            def fold_mm(b):
                tensor.wait_ge(sACT, ACT[f"WROW{b}"])
                for dc in range(8):
                    mm = tensor.matmul(
                        out=ps_small[:, 8 + dc:9 + dc],
                        lhsT=w_row[b][0:1, dc * 128:(dc + 1) * 128],
                        rhs=ones_bf[0:1, 0:1],
                        start=True, stop=True,
                    )
                mm.then_inc(sPE, 1)                      # FOLD{b}

            def proj_mm(b):
                tensor.wait_ge(sDVE, DVE[f"WCOL{b}"])
                if b == 0:
                    tensor.wait_ge(sWV, 16)
                    tensor.wait_ge(sACT, ACT["QTSB1"])   # ps_a overwrite guard
                for dc in range(8):
                    for nh in range(2):
                        mm = tensor.matmul(
                            out=ps_a[0:1, nh * 512:(nh + 1) * 512],
                            lhsT=w_col[b][:, dc:dc + 1],
                            rhs=wv_sb[:, dc, nh * 512:(nh + 1) * 512],
                            start=(dc == 0), stop=(dc == 7),
                        )
                mm.then_inc(sPE, 1)                      # PROJ{b}

            # batch-0 tail (fold0/proj0) threads through w1's DMA-pacing gaps
            z_mm(0)
            for c in range(NCHUNK):
                w_chunk(0, c)
            z_mm(1)
            w_chunk(1, 0)
            fold_mm(0)
            w_chunk(1, 1)
            proj_mm(0)
            w_chunk(1, 2)
            w_chunk(1, 3)
            fold_mm(1)
            proj_mm(1)

        # ---------- DVE ----------
        @blk.vector
        def _(vector):
            vector.memset(ones_col[:], 1.0).then_inc(sDVE, 1)
            vector.memset(ones_bf[:], 1.0).then_inc(sDVE, 1)

            def qcol(b):
                vector.wait_ge(sPE, PE[f"QF{b}"])
                vector.tensor_copy(out=qcol_sb[b][:], in_=ps_small[:, 16:24]) \
                    .then_inc(sDVE, 1)                                # QCOL{b}

            def small_chain(b, step):
                if step == 0:
                    if b == 0:
                        vector.wait_ge(sW, 64)
                    vector.wait_ge(sPE, PE[f"QPROW{b}"])
                    vector.tensor_add(qpr_sb[b][:], ps_small[0:1, 256:256 + A],
                                      bkq_row[:]).then_inc(sDVE, 1)   # QPRSB{b}
                elif step == 1:
                    vector.wait_ge(sPE, PE[f"QPF{b}"])
                    vector.tensor_copy(out=qp_sb[b][:], in_=ps_small[:, 0:2]) \
                        .then_inc(sDVE, 1)                            # QPSB{b}
                else:
                    vector.wait_ge(sPE, PE[f"QTB{b}"])
                    vector.tensor_copy(out=qtb_sb[b][:], in_=ps_b[:]) \
                        .then_inc(sDVE, 1)                            # QTBSB{b}

            def mult_chunk(b, c):
                g = b * NCHUNK + c
                vector.wait_ge(sK[g % KBUFS], (g // KBUFS + 1) * 16)
                if c == 0:
                    # self-wait: qtb_sb copy completion before reads
                    vector.wait_ge(sDVE, DVE[f"QTBSB{b}"])
                for j in range(CHUNK):
                    t = c * CHUNK + j
                    if j < NACT:
                        if g >= 1:
                            # scratch slot j: previous chunk's reduce done
                            bp, cp = divmod(g - 1, NCHUNK)
                            vector.wait_ge(sACT, ACT[red_ev(bp, cp, j)])
                        vector.tensor_mul(
                            scr[j][:], kt[g % KBUFS][:, j, :], qtb_sb[b][:]
                        ).then_inc(sDVE, 1)               # MUL{b}_{c}_{j}
                    else:
                        if g >= 1:
                            # junk WAW: self-wait (always satisfied in-order;
                            # appeases the address-level race detector)
                            bp, cp = divmod(g - 1, NCHUNK)
                            vector.wait_ge(sDVE, DVE[mult_ev(bp, cp, j)])
                        # fused dot product on DVE: out=(k*1)*qt, accum=row sum
                        vector.scalar_tensor_tensor(
                            out=junk[j - NACT][:],
                            in0=kt[g % KBUFS][:, j, :], scalar=1.0,
                            in1=qtb_sb[b][:],
                            op0=AL.mult, op1=AL.mult,
                            accum_out=smat[b][:, t:t + 1],
                        ).then_inc(sDVE, 1)               # MUL{b}_{c}_{j}

            def tail(b, step):
                if step == 0:
                    vector.wait_ge(sPE, PE[f"Z{b}"])
                    vector.reciprocal(invz[b][:], ps_small[0:1, 4:5]) \
                        .then_inc(sDVE, 1)                            # INVZ{b}
                elif step == 1:
                    vector.wait_ge(sPE, PE[f"FOLD{b}"])
                    vector.tensor_copy(out=w_col[b][:], in_=ps_small[:, 8:16]) \
                        .then_inc(sDVE, 1)                            # WCOL{b}
                else:
                    vector.wait_ge(sPE, PE[f"PROJ{b}"])
                    if b == 0:
                        vector.wait_ge(sBV, 16)
                    vector.tensor_add(o_sb[b][:], ps_a[0:1, :], bv_row[:]) \
                        .then_inc(sDVE, 1)                            # OSB{b}

            qcol(0)
            small_chain(0, 0)
            qcol(1)
            small_chain(0, 1)
            small_chain(0, 2)
            mult_chunk(0, 0)
            small_chain(1, 0)
            mult_chunk(0, 1)
            small_chain(1, 1)
            mult_chunk(0, 2)
            small_chain(1, 2)
            mult_chunk(0, 3)
            mult_chunk(1, 0)
            tail(0, 0)          # INVZ0
            mult_chunk(1, 1)
            mult_chunk(1, 2)
            mult_chunk(1, 3)
            tail(0, 1)          # WCOL0
            tail(0, 2)          # OSB0
            tail(1, 0)
            tail(1, 1)
            tail(1, 2)

        # ---------- ACT (scalar) ----------
        @blk.scalar
        def _(scalar):
            def qtsb(b):
                scalar.wait_ge(sPE, PE[f"QT{b}"])
                scalar.mul(qt_sb[b][:], ps_a[0:1, :], 1.0 / 16.0) \
                    .then_inc(sACT, 1)                                # QTSB{b}

            def red_chunk(b, c):
                for j in range(NACT):
                    t = c * CHUNK + j
                    scalar.wait_ge(sDVE, DVE[mult_ev(b, c, j)])
                    scalar.activation(
                        out=scr[j][:], in_=scr[j][:], func=AF.Copy,
                        accum_out=smat[b][:, t:t + 1],
                    ).then_inc(sACT, 1)                   # RED{b}_{c}_{j}

            def expb(b):
                # smat writers: ACT reduces (self-order) + DVE fused stts
                scalar.wait_ge(sACT, ACT[red_ev(b, NCHUNK - 1, NACT - 1)])
                scalar.wait_ge(sDVE, DVE[mult_ev(b, NCHUNK - 1, CHUNK - 1)])
                scalar.activation(
                    out=pmat[b][:], in_=smat[b][:], func=AF.Exp,
                    accum_out=psums[b][:],
                ).then_inc(sACT, 1)                                   # EXP{b}

            def wrow(b):
                scalar.wait_ge(sPE, PE[f"W{b}C{NCHUNK - 1}"])
                scalar.wait_ge(sDVE, DVE[f"INVZ{b}"])
                scalar.activation(
                    out=w_row[b][:], in_=ps_w[0:1, :], func=AF.Copy,
                    bias=0.0, scale=invz[b][0:1, 0:1],
                ).then_inc(sACT, 1)                                   # WROW{b}

            qtsb(0)
            red_chunk(0, 0)
            qtsb(1)
            for c in range(1, NCHUNK):
                red_chunk(0, c)
            expb(0)
            for c in range(NCHUNK):
                red_chunk(1, c)
            expb(1)
            wrow(0)
            wrow(1)

    return nc


_NC_CACHE = None


def get_nc():
    global _NC_CACHE
    if _NC_CACHE is None:
        _NC_CACHE = _build_nc()
    return _NC_CACHE


def make_in_maps(q, k, v, W_kq, b_kq, W_v, b_v):
    """Shard full inputs over 8 cores: batch-parallel, weights replicated.
    k, v, W_v are cast to bfloat16 on the host (compute dtype of the
    streaming contractions)."""
    import ml_dtypes

    bf16 = ml_dtypes.bfloat16
    q = np.ascontiguousarray(
        np.asarray(q, dtype=np.float32).reshape(B, E).astype(bf16))
    k = np.ascontiguousarray(np.asarray(k, dtype=np.float32).astype(bf16))
    v = np.ascontiguousarray(np.asarray(v, dtype=np.float32).astype(bf16))
    W_kq32 = np.asarray(W_kq, dtype=np.float32)
    W_kq = np.ascontiguousarray(W_kq32.astype(bf16))
    W_kqT = np.ascontiguousarray(W_kq32.T.astype(bf16))
    b_kq = np.ascontiguousarray(np.asarray(b_kq, dtype=np.float32))
    W_v = np.ascontiguousarray(np.asarray(W_v, dtype=np.float32).astype(bf16))
    b_v = np.ascontiguousarray(np.asarray(b_v, dtype=np.float32))
    in_maps = []
    for i in range(NCORES):
        lo, hi = i * BPC, (i + 1) * BPC
        in_maps.append({
            "q": q[lo:hi],
            "k": k[lo:hi],
            "v": v[lo:hi],
            "W_kq": W_kq,
            "W_kqT": W_kqT,
            "b_kq": b_kq,
            "W_v": W_v,
            "b_v": b_v,
        })
    return in_maps


def kernel(q, k, v, W_kq, b_kq, W_v, b_v):
    from concourse.bass_utils import run_bass_kernel_spmd

    nc = get_nc()
    in_maps = make_in_maps(q, k, v, W_kq, b_kq, W_v, b_v)
    res = run_bass_kernel_spmd(nc, in_maps, core_ids=list(range(NCORES)))
    out = np.concatenate([res.results[i]["out"] for i in range(NCORES)], axis=0)
    return np.ascontiguousarray(out.astype(np.float32))


# revision 47
# speedup vs baseline: 1.0402x; 1.0402x over previous
"""Distributed attention kernel for Trainium2 (8 NeuronCores, SPMD).

Problem: B=16 batches of single-query attention over NK=4096 keys,
EMBED=1024, ATTN=256, with a shared kq projection and a v projection.

Math restructuring (exact up to float reassociation):
  - scores = (q@W_kq + b_kq) @ (k@W_kq + b_kq)^T / 16
           = k @ qt + const            where qt = W_kq @ (W_kq^T q + b_kq) / 16
    (the constant offsets every score equally -> softmax invariant, dropped)
  - out = softmax(scores) @ (v@W_v + b_v)
        = (attn @ v) @ W_v + b_v       (attn sums to 1)
This removes the O(NK*E*E) v-projection and O(NK*E*A) k-projection
entirely; the kernel is HBM-bandwidth bound streaming k and v once.

Sharding: data-parallel over batch, 2 batches per core. k, v, W_v are
cast to bf16 on the host (compute dtype of the streaming contractions,
halves HBM traffic); W_kq is additionally passed pre-transposed.

Token layout is p-major ("(p s) d"): partition p holds NSUB consecutive
token rows, so each chunk DMA is one 16KB-contiguous run per partition
(128 descriptors, not 1024). k and v use the same permutation, and
softmax is globally permutation-invariant, so results are unchanged.

s = k.qt is load-balanced across engines per 128-token tile (per chunk
of 8 tiles: 5 via DVE bf16 multiply (2x mode) + ACT activation-accum
row-sum; 3 via DVE fused scalar_tensor_tensor). w = attn@v and the
projections run on TensorE in bf16 (single pass). Softmax uses
unnormalized exp (scores ~ N(0,1), no overflow in fp32) with 1/Z
folded into the w_row evacuation. Batch 1's DMAs are interleaved with
batch 0's v stream so its k chunks are resident when compute frees up.

Raw bass (not Tile): this toolchain's walrus build rejects >1 embedded
sync-wait per compute instruction, which Tile's scheduler emits; raw
bass uses standalone sequencer waits, with explicit semaphore ticks
precomputed in python (the *_seq tables below).

PSUM bank map (PE-W vs DVE/ACT-R hazards serialized via the sem chain;
bank granularity, not address: concurrent PE-write + DVE/ACT-read of
the SAME bank is a fatal HW error even at different addresses):
  bank 0    : qp_row [0:1,256:512], qp_col [:,0:2], Z [0:1,4:5],
              w fold [:,8:16], q fold [:,16:24]
  banks 1-2 : qt row / out row (disjoint lifetimes)
  banks 3-4 : qt broadcast [128, 1024]
  banks 5-6 : w accumulator row
"""

import contextlib

import numpy as np

try:
    import concourse.bass as bass  # noqa: F401
except ImportError:  # fallback if site path isn't preloaded
    import sys

    sys.path.insert(0, "/opt/trn_rl_repo")

B = 16
NCORES = 8
BPC = B // NCORES  # batches per core
NK = 4096
E = 1024
A = 256
NSUB = NK // 128   # 32 token subtiles of 128
CHUNK = 8          # subtiles per DMA chunk (2 MB in bf16)
NCHUNK = NSUB // CHUNK
KBUFS = 3
VBUFS = 4
NACT = 5           # tiles per chunk reduced on ACT (rest fused on DVE)
# scratch slot j is dedicated to tile position j of each chunk, so a
# chunk's multiplies only wait on the PREVIOUS chunk's reduces


def _build_nc():
    import concourse.bass as bass
    from concourse import mybir

    FP = mybir.dt.float32
    BF = mybir.dt.bfloat16
    AL = mybir.AluOpType
    AF = mybir.ActivationFunctionType

    nc = bass.Bass()
    q_d = nc.declare_dram_parameter("q", [BPC, E], BF, isOutput=False)
    k_d = nc.declare_dram_parameter("k", [BPC, NK, E], BF, isOutput=False)
    v_d = nc.declare_dram_parameter("v", [BPC, NK, E], BF, isOutput=False)
    wkq_d = nc.declare_dram_parameter("W_kq", [E, A], BF, isOutput=False)
    wkqT_d = nc.declare_dram_parameter("W_kqT", [A, E], BF, isOutput=False)
    bkq_d = nc.declare_dram_parameter("b_kq", [A], FP, isOutput=False)
    wv_d = nc.declare_dram_parameter("W_v", [E, E], BF, isOutput=False)
    bv_d = nc.declare_dram_parameter("b_v", [E], FP, isOutput=False)
    out_d = nc.declare_dram_parameter("out", [BPC, E], FP, isOutput=True)

    with contextlib.ExitStack() as st:
        def sb(name, shape, dt=FP):
            return st.enter_context(nc.sbuf_tensor(name, shape, dt))

        # ---- SBUF ----
        wkq_sb = sb("wkq_sb", [128, 8, A], BF)
        wkqT_sb = sb("wkqT_sb", [128, 2, E], BF)
        wv_sb = sb("wv_sb", [128, 8, E], BF)
        q_row = sb("q_row", [1, BPC * E], BF)
        qcol_sb = [sb(f"qcol_sb{b}", [128, 8], BF) for b in range(BPC)]
        bkq_row = sb("bkq_row", [1, A])
        bv_row = sb("bv_row", [1, E])
        ones_col = sb("ones_col", [128, 1])        # fp32 (Z rhs)
        ones_bf = sb("ones_bf", [1, 128], BF)      # bf16 (w fold rhs)
        kt = [sb(f"kt{i}", [128, CHUNK, E], BF) for i in range(KBUFS)]
        vt = [sb(f"vt{i}", [128, CHUNK, E], BF) for i in range(VBUFS)]
        scr = [sb(f"scr{i}", [128, E], BF) for i in range(NACT)]
        junk = [sb(f"junk{i}", [128, E], BF) for i in range(CHUNK - NACT)]
        # per-batch smalls
        qpr_sb = [sb(f"qpr_sb{b}", [1, A], BF) for b in range(BPC)]
        qp_sb = [sb(f"qp_sb{b}", [128, 2], BF) for b in range(BPC)]
        qt_sb = [sb(f"qt_sb{b}", [1, E], BF) for b in range(BPC)]
        qtb_sb = [sb(f"qtb_sb{b}", [128, E], BF) for b in range(BPC)]
        smat = [sb(f"smat{b}", [128, NSUB]) for b in range(BPC)]
        pmat = [sb(f"pmat{b}", [128, NSUB], BF) for b in range(BPC)]
        psums = [sb(f"psums{b}", [128, 1]) for b in range(BPC)]
        invz = [sb(f"invz{b}", [1, 1]) for b in range(BPC)]
        w_row = [sb(f"w_row{b}", [1, E], BF) for b in range(BPC)]
        w_col = [sb(f"w_col{b}", [128, 8], BF) for b in range(BPC)]
        o_sb = [sb(f"o_sb{b}", [1, E]) for b in range(BPC)]

        # ---- PSUM (static bank map) ----
        ps_small = st.enter_context(nc.psum_tensor([128, 512], FP))   # bank 0
        ps_a = st.enter_context(nc.psum_tensor([128, 1024], FP))      # banks 1-2
        ps_b = st.enter_context(nc.psum_tensor([128, 1024], FP))      # banks 3-4
        ps_w = st.enter_context(nc.psum_tensor([128, 1024], FP))      # banks 5-6

        # ---- semaphores ----
        sW = st.enter_context(nc.semaphore("sW"))      # wkq+wkqT+q+bkq -> 64
        sWV = st.enter_context(nc.semaphore("sWV"))    # wv -> 16
        sBV = st.enter_context(nc.semaphore("sBV"))    # bv -> 16
        sK = [st.enter_context(nc.semaphore(f"sK{i}")) for i in range(KBUFS)]
        sV = [st.enter_context(nc.semaphore(f"sV{i}")) for i in range(VBUFS)]
        sOUT = st.enter_context(nc.semaphore("sOUT"))
        sPE = st.enter_context(nc.semaphore("sPE"))
        sDVE = st.enter_context(nc.semaphore("sDVE"))
        sACT = st.enter_context(nc.semaphore("sACT"))

        blk = st.enter_context(nc.Block())

        # ---------- event tick registry ----------
        def ticks(seq):
            return {ev: i + 1 for i, ev in enumerate(seq)}

        pe_seq = ["QF0", "QPROW0", "QF1", "QPF0", "QT0", "QTB0",
                  "QPROW1", "QPF1", "QT1", "QTB1"]
        pe_seq += ["Z0"] + [f"W0C{c}" for c in range(NCHUNK)]
        pe_seq += ["Z1", "W1C0", "FOLD0", "W1C1", "PROJ0", "W1C2", "W1C3",
                   "FOLD1", "PROJ1"]
        PE = ticks(pe_seq)

        def mult_ev(b, c, j):
            # DVE inc for tile j of chunk (b, c): mult (j < NACT) or fused stt
            return f"MUL{b}_{c}_{j}"

        def red_ev(b, c, j):
            return f"RED{b}_{c}_{j}"

        dve_seq = ["MS1", "MS2", "QCOL0", "QPRSB0", "QCOL1", "QPSB0",
                   "QTBSB0"]
        dve_seq += [mult_ev(0, 0, j) for j in range(CHUNK)]
        dve_seq += ["QPRSB1"]
        dve_seq += [mult_ev(0, 1, j) for j in range(CHUNK)]
        dve_seq += ["QPSB1"]
        dve_seq += [mult_ev(0, 2, j) for j in range(CHUNK)]
        dve_seq += ["QTBSB1"]
        dve_seq += [mult_ev(0, 3, j) for j in range(CHUNK)]
        dve_seq += [mult_ev(1, 0, j) for j in range(CHUNK)]
        dve_seq += ["INVZ0"]
        dve_seq += [mult_ev(1, 1, j) for j in range(CHUNK)]
        dve_seq += [mult_ev(1, 2, j) for j in range(CHUNK)]
        dve_seq += [mult_ev(1, 3, j) for j in range(CHUNK)]
        dve_seq += ["WCOL0", "OSB0", "INVZ1", "WCOL1", "OSB1"]
        DVE = ticks(dve_seq)

        act_seq = ["QTSB0"]
        act_seq += [red_ev(0, 0, j) for j in range(NACT)]
        act_seq += ["QTSB1"]
        for c in range(1, NCHUNK):
            act_seq += [red_ev(0, c, j) for j in range(NACT)]
        act_seq += ["EXP0"]
        for c in range(NCHUNK):
            act_seq += [red_ev(1, c, j) for j in range(NACT)]
        act_seq += ["EXP1", "WROW0", "WROW1"]
        ACT = ticks(act_seq)

        # ---------- SYNC: all DMAs ----------
        @blk.sync
        def _(sync):
            sync.dma_start(
                out=wkq_sb[:], in_=wkq_d[:].rearrange("(dc p) a -> p dc a", p=128)
            ).then_inc(sW, 16)
            sync.dma_start(
                out=wkqT_sb[:], in_=wkqT_d[:].rearrange("(ac p) d -> p ac d", p=128)
            ).then_inc(sW, 16)
            sync.dma_start(
                out=q_row[:], in_=q_d[:].rearrange("b e -> (b e)")[None, :]
            ).then_inc(sW, 16)
            sync.dma_start(out=bkq_row[:], in_=bkq_d[:][None, :]).then_inc(sW, 16)
            sync.dma_start(out=bv_row[:], in_=bv_d[:][None, :]).then_inc(sBV, 16)

            def kdma(b, c):
                g = b * NCHUNK + c
                if g >= KBUFS:
                    gp = g - KBUFS
                    bp, cp = divmod(gp, NCHUNK)
                    sync.wait_ge(sDVE, DVE[mult_ev(bp, cp, CHUNK - 1)])
                k_b = k_d[:][b].rearrange("(p s) d -> p s d", p=128)
                sync.dma_start(
                    out=kt[g % KBUFS][:],
                    in_=k_b[:, c * CHUNK:(c + 1) * CHUNK, :],
                ).then_inc(sK[g % KBUFS], 16)

            def vdma(b, c):
                g = b * NCHUNK + c
                if g >= VBUFS:
                    gp = g - VBUFS
                    bp, cp = divmod(gp, NCHUNK)
                    sync.wait_ge(sPE, PE[f"W{bp}C{cp}"])
                v_b = v_d[:][b].rearrange("(p s) d -> p s d", p=128)
                sync.dma_start(
                    out=vt[g % VBUFS][:],
                    in_=v_b[:, c * CHUNK:(c + 1) * CHUNK, :],
                ).then_inc(sV[g % VBUFS], 16)

            # k1 interleaved with v0 so it streams during k0/v0 compute;
            # PE-gated issues (v0c3, k1c3) pushed late to avoid head-of-line
            for c in range(NCHUNK):
                kdma(0, c)
            sync.dma_start(
                out=wv_sb[:], in_=wv_d[:].rearrange("(dc p) e -> p dc e", p=128)
            ).then_inc(sWV, 16)
            vdma(0, 0)
            vdma(0, 1)
            vdma(0, 2)
            kdma(1, 0)
            kdma(1, 1)
            vdma(0, 3)
            kdma(1, 2)
            kdma(1, 3)
            for c in range(NCHUNK):
                vdma(1, c)

            for b in range(BPC):
                sync.wait_ge(sDVE, DVE[f"OSB{b}"])
                sync.dma_start(out=out_d[:][b:b + 1, :], in_=o_sb[b][:]).then_inc(
                    sOUT, 16)
            sync.wait_ge(sOUT, BPC * 16)

        # ---------- PE ----------
        @blk.tensor
        def _(tensor):
            tensor.wait_ge(sW, 64)
            tensor.wait_ge(sDVE, DVE["MS2"])  # ones tiles ready
            def qfold(b):
                if b > 0:
                    # bank-0 serialization: latest possible concurrent reader
                    tensor.wait_ge(sDVE, DVE[f"QPRSB{b - 1}"])
                for dc in range(8):
                    mm = tensor.matmul(
                        out=ps_small[:, 16 + dc:17 + dc],
                        lhsT=q_row[0:1, b * E + dc * 128:b * E + (dc + 1) * 128],
                        rhs=ones_bf[0:1, 0:1],
                        start=True, stop=True,
                    )
                mm.then_inc(sPE, 1)                      # QF{b}

            def qprow(b):
                tensor.wait_ge(sDVE, DVE[f"QCOL{b}"])
                if b > 0:
                    # bank-0 safety: prior batch's bank-0 reads done
                    tensor.wait_ge(sDVE, DVE[f"QPSB{b - 1}"])
                for dc in range(8):
                    mm = tensor.matmul(
                        out=ps_small[0:1, 256:256 + A],
                        lhsT=qcol_sb[b][:, dc:dc + 1],
                        rhs=wkq_sb[:, dc, :],
                        start=(dc == 0), stop=(dc == 7),
                    )
                mm.then_inc(sPE, 1)                      # QPROW{b}

            def qpf(b):
                tensor.wait_ge(sDVE, DVE[f"QPRSB{b}"])
                if b == 0:
                    # bank-0 serialization vs QCOL1's read
                    tensor.wait_ge(sDVE, DVE["QCOL1"])
                for c2 in range(2):
                    mm = tensor.matmul(
                        out=ps_small[:, c2:c2 + 1],
                        lhsT=qpr_sb[b][0:1, c2 * 128:(c2 + 1) * 128],
                        rhs=ones_bf[0:1, 0:1],
                        start=True, stop=True,
                    )
                mm.then_inc(sPE, 1)                      # QPF{b}

            def qt_mm(b):
                tensor.wait_ge(sDVE, DVE[f"QPSB{b}"])
                if b > 0:
                    tensor.wait_ge(sACT, ACT[f"QTSB{b - 1}"])
                for ac in range(2):
                    for nh in range(2):
                        mm = tensor.matmul(
                            out=ps_a[0:1, nh * 512:(nh + 1) * 512],
                            lhsT=qp_sb[b][:, ac:ac + 1],
                            rhs=wkqT_sb[:, ac, nh * 512:(nh + 1) * 512],
                            start=(ac == 0), stop=(ac == 1),
                        )
                mm.then_inc(sPE, 1)                      # QT{b}

            def qtb_mm(b):
                tensor.wait_ge(sACT, ACT[f"QTSB{b}"])
                for nh in range(2):
                    mm = tensor.matmul(
                        out=ps_b[:, nh * 512:(nh + 1) * 512],
                        lhsT=ones_bf[:],
                        rhs=qt_sb[b][0:1, nh * 512:(nh + 1) * 512],
                        start=True, stop=True,
                    )
                mm.then_inc(sPE, 1)                      # QTB{b}

            qfold(0)
            qprow(0)
            qfold(1)
            qpf(0)
            qt_mm(0)
            qtb_mm(0)
            qprow(1)
            qpf(1)
            qt_mm(1)
            qtb_mm(1)

            def z_mm(b):
                tensor.wait_ge(sACT, ACT[f"EXP{b}"])
                tensor.matmul(
                    out=ps_small[0:1, 4:5], lhsT=psums[b][:], rhs=ones_col[:],
                    start=True, stop=True,
                ).then_inc(sPE, 1)                       # Z{b}

            def w_chunk(b, c):
                g = b * NCHUNK + c
                tensor.wait_ge(sV[g % VBUFS], (g // VBUFS + 1) * 16)
                if b == 1 and c == 0:
                    # ps_w WAR: batch-0 w_row evacuation must complete
                    tensor.wait_ge(sACT, ACT["WROW0"])
                for j in range(CHUNK):
                    t = c * CHUNK + j
                    for nh in range(2):
                        mm = tensor.matmul(
                            out=ps_w[0:1, nh * 512:(nh + 1) * 512],
                            lhsT=pmat[b][:, t:t + 1],
                            rhs=vt[g % VBUFS][:, j, nh * 512:(nh + 1) * 512],
                            start=(t == 0), stop=(t == NSUB - 1),
                        )
                mm.then_inc(sPE, 1)                      # W{b}C{c}

            def fold_mm(b):
                tensor.wait_ge(sACT, ACT[f"WROW{b}"])
                for dc in range(8):
                    mm = tensor.matmul(
                        out=ps_small[:, 8 + dc:9 + dc],
                        lhsT=w_row[b][0:1, dc * 128:(dc + 1) * 128],
                        rhs=ones_bf[0:1, 0:1],
                        start=True, stop=True,
                    )
                mm.then_inc(sPE, 1)                      # FOLD{b}

            def proj_mm(b):
                tensor.wait_ge(sDVE, DVE[f"WCOL{b}"])
                if b == 0:
                    tensor.wait_ge(sWV, 16)
                    tensor.wait_ge(sACT, ACT["QTSB1"])   # ps_a overwrite guard
                for dc in range(8):
                    for nh in range(2):
                        mm = tensor.matmul(
                            out=ps_a[0:1, nh * 512:(nh + 1) * 512],
                            lhsT=w_col[b][:, dc:dc + 1],
                            rhs=wv_sb[:, dc, nh * 512:(nh + 1) * 512],
                            start=(dc == 0), stop=(dc == 7),
                        )
                mm.then_inc(sPE, 1)                      # PROJ{b}

            # batch-0 tail (fold0/proj0) threads through w1's DMA-pacing gaps
            z_mm(0)
            for c in range(NCHUNK):
                w_chunk(0, c)
            z_mm(1)
            w_chunk(1, 0)
            fold_mm(0)
            w_chunk(1, 1)
            proj_mm(0)
            w_chunk(1, 2)
            w_chunk(1, 3)
            fold_mm(1)
            proj_mm(1)

        # ---------- DVE ----------
        @blk.vector
        def _(vector):
            vector.memset(ones_col[:], 1.0).then_inc(sDVE, 1)
            vector.memset(ones_bf[:], 1.0).then_inc(sDVE, 1)

            def qcol(b):
                vector.wait_ge(sPE, PE[f"QF{b}"])
                vector.tensor_copy(out=qcol_sb[b][:], in_=ps_small[:, 16:24]) \
                    .then_inc(sDVE, 1)                                # QCOL{b}

            def small_chain(b, step):
                if step == 0:
                    if b == 0:
                        vector.wait_ge(sW, 64)
                    vector.wait_ge(sPE, PE[f"QPROW{b}"])
                    vector.tensor_add(qpr_sb[b][:], ps_small[0:1, 256:256 + A],
                                      bkq_row[:]).then_inc(sDVE, 1)   # QPRSB{b}
                elif step == 1:
                    vector.wait_ge(sPE, PE[f"QPF{b}"])
                    vector.tensor_copy(out=qp_sb[b][:], in_=ps_small[:, 0:2]) \
                        .then_inc(sDVE, 1)                            # QPSB{b}
                else:
                    vector.wait_ge(sPE, PE[f"QTB{b}"])
                    vector.tensor_copy(out=qtb_sb[b][:], in_=ps_b[:]) \
                        .then_inc(sDVE, 1)                            # QTBSB{b}

            def mult_chunk(b, c):
                g = b * NCHUNK + c
                vector.wait_ge(sK[g % KBUFS], (g // KBUFS + 1) * 16)
                if c == 0:
                    # self-wait: qtb_sb copy completion before reads
                    vector.wait_ge(sDVE, DVE[f"QTBSB{b}"])
                for j in range(CHUNK):
                    t = c * CHUNK + j
                    if j < NACT:
                        if g >= 1:
                            # scratch slot j: previous chunk's reduce done
                            bp, cp = divmod(g - 1, NCHUNK)
                            vector.wait_ge(sACT, ACT[red_ev(bp, cp, j)])
                        vector.tensor_mul(
                            scr[j][:], kt[g % KBUFS][:, j, :], qtb_sb[b][:]
                        ).then_inc(sDVE, 1)               # MUL{b}_{c}_{j}
                    else:
                        if g >= 1:
                            # junk WAW: self-wait (always satisfied in-order;
                            # appeases the address-level race detector)
                            bp, cp = divmod(g - 1, NCHUNK)
                            vector.wait_ge(sDVE, DVE[mult_ev(bp, cp, j)])
                        # fused dot product on DVE: out=(k*1)*qt, accum=row sum
                        vector.scalar_tensor_tensor(
                            out=junk[j - NACT][:],
                            in0=kt[g % KBUFS][:, j, :], scalar=1.0,
                            in1=qtb_sb[b][:],
                            op0=AL.mult, op1=AL.mult,
                            accum_out=smat[b][:, t:t + 1],
                        ).then_inc(sDVE, 1)               # MUL{b}_{c}_{j}

            def tail(b, step):
                if step == 0:
                    vector.wait_ge(sPE, PE[f"Z{b}"])
                    vector.reciprocal(invz[b][:], ps_small[0:1, 4:5]) \
                        .then_inc(sDVE, 1)                            # INVZ{b}
                elif step == 1:
                    vector.wait_ge(sPE, PE[f"FOLD{b}"])
                    vector.tensor_copy(out=w_col[b][:], in_=ps_small[:, 8:16]) \
                        .then_inc(sDVE, 1)                            # WCOL{b}
                else:
                    vector.wait_ge(sPE, PE[f"PROJ{b}"])
                    if b == 0:
                        vector.wait_ge(sBV, 16)
                    vector.tensor_add(o_sb[b][:], ps_a[0:1, :], bv_row[:]) \
                        .then_inc(sDVE, 1)                            # OSB{b}

            qcol(0)
            small_chain(0, 0)
            qcol(1)
            small_chain(0, 1)
            small_chain(0, 2)
            mult_chunk(0, 0)
            small_chain(1, 0)
            mult_chunk(0, 1)
            small_chain(1, 1)
            mult_chunk(0, 2)
            small_chain(1, 2)
            mult_chunk(0, 3)
            mult_chunk(1, 0)
            tail(0, 0)          # INVZ0
            mult_chunk(1, 1)
            mult_chunk(1, 2)
            mult_chunk(1, 3)
            tail(0, 1)          # WCOL0
            tail(0, 2)          # OSB0
            tail(1, 0)
            tail(1, 1)
            tail(1, 2)

        # ---------- ACT (scalar) ----------
        @blk.scalar
        def _(scalar):
            def qtsb(b):
                scalar.wait_ge(sPE, PE[f"QT{b}"])
                scalar.mul(qt_sb[b][:], ps_a[0:1, :], 1.0 / 16.0) \
                    .then_inc(sACT, 1)                                # QTSB{b}

            def red_chunk(b, c):
                for j in range(NACT):
                    t = c * CHUNK + j
                    scalar.wait_ge(sDVE, DVE[mult_ev(b, c, j)])
                    scalar.activation(
                        out=scr[j][:], in_=scr[j][:], func=AF.Copy,
                        accum_out=smat[b][:, t:t + 1],
                    ).then_inc(sACT, 1)                   # RED{b}_{c}_{j}

            def expb(b):
                # smat writers: ACT reduces (self-order) + DVE fused stts
                scalar.wait_ge(sACT, ACT[red_ev(b, NCHUNK - 1, NACT - 1)])
                scalar.wait_ge(sDVE, DVE[mult_ev(b, NCHUNK - 1, CHUNK - 1)])
                scalar.activation(
                    out=pmat[b][:], in_=smat[b][:], func=AF.Exp,
                    accum_out=psums[b][:],
                ).then_inc(sACT, 1)                                   # EXP{b}

            def wrow(b):
                scalar.wait_ge(sPE, PE[f"W{b}C{NCHUNK - 1}"])
                scalar.wait_ge(sDVE, DVE[f"INVZ{b}"])
                scalar.activation(
                    out=w_row[b][:], in_=ps_w[0:1, :], func=AF.Copy,
                    bias=0.0, scale=invz[b][0:1, 0:1],
                ).then_inc(sACT, 1)                                   # WROW{b}

            qtsb(0)
            red_chunk(0, 0)
            qtsb(1)
            for c in range(1, NCHUNK):
                red_chunk(0, c)
            expb(0)
            for c in range(NCHUNK):
                red_chunk(1, c)
            expb(1)
            wrow(0)
            wrow(1)

    return nc


_NC_CACHE = None


def get_nc():
    global _NC_CACHE
    if _NC_CACHE is None:
        _NC_CACHE = _build_nc()
    return _NC_CACHE


def make_in_maps(q, k, v, W_kq, b_kq, W_v, b_v):
    """Shard full inputs over 8 cores: batch-parallel, weights replicated.
    k, v, W_v are cast to bfloat16 on the host (compute dtype of the
    streaming contractions)."""
    import ml_dtypes

    bf16 = ml_dtypes.bfloat16
    q = np.ascontiguousarray(
        np.asarray(q, dtype=np.float32).reshape(B, E).astype(bf16))
    k = np.ascontiguousarray(np.asarray(k, dtype=np.float32).astype(bf16))
    v = np.ascontiguousarray(np.asarray(v, dtype=np.float32).astype(bf16))
    W_kq32 = np.asarray(W_kq, dtype=np.float32)
    W_kq = np.ascontiguousarray(W_kq32.astype(bf16))
    W_kqT = np.ascontiguousarray(W_kq32.T.astype(bf16))
    b_kq = np.ascontiguousarray(np.asarray(b_kq, dtype=np.float32))
    W_v = np.ascontiguousarray(np.asarray(W_v, dtype=np.float32).astype(bf16))
    b_v = np.ascontiguousarray(np.asarray(b_v, dtype=np.float32))
    in_maps = []
    for i in range(NCORES):
        lo, hi = i * BPC, (i + 1) * BPC
        in_maps.append({
            "q": q[lo:hi],
            "k": k[lo:hi],
            "v": v[lo:hi],
            "W_kq": W_kq,
            "W_kqT": W_kqT,
            "b_kq": b_kq,
            "W_v": W_v,
            "b_v": b_v,
        })
    return in_maps


def kernel(q, k, v, W_kq, b_kq, W_v, b_v):
    from concourse.bass_utils import run_bass_kernel_spmd

    nc = get_nc()
    in_maps = make_in_maps(q, k, v, W_kq, b_kq, W_v, b_v)
    res = run_bass_kernel_spmd(nc, in_maps, core_ids=list(range(NCORES)))
    out = np.concatenate([res.results[i]["out"] for i in range(NCORES)], axis=0)
    return np.ascontiguousarray(out.astype(np.float32))


# revision 48
# speedup vs baseline: 1.0849x; 1.0430x over previous
"""Distributed attention kernel for Trainium2 (8 NeuronCores, SPMD).

Problem: B=16 batches of single-query attention over NK=4096 keys,
EMBED=1024, ATTN=256, with a shared kq projection and a v projection.

Math restructuring (exact up to float reassociation):
  - scores = (q@W_kq + b_kq) @ (k@W_kq + b_kq)^T / 16
           = k @ qt + const            where qt = W_kq @ (W_kq^T q + b_kq) / 16
    (the constant offsets every score equally -> softmax invariant, dropped)
  - out = softmax(scores) @ (v@W_v + b_v)
        = (attn @ v) @ W_v + b_v       (attn sums to 1)
This removes the O(NK*E*E) v-projection and O(NK*E*A) k-projection
entirely; the kernel is HBM-bandwidth bound streaming k and v once.

Sharding: data-parallel over batch, 2 batches per core. k, v, W_v are
cast to bf16 on the host (compute dtype of the streaming contractions,
halves HBM traffic); W_kq is additionally passed pre-transposed.

Token layout is p-major ("(p s) d"): partition p holds NSUB consecutive
token rows, so each chunk DMA is one 16KB-contiguous run per partition
(128 descriptors, not 1024). k and v use the same permutation, and
softmax is globally permutation-invariant, so results are unchanged.

s = k.qt is load-balanced across engines per 128-token tile (per chunk
of 8 tiles: 5 via DVE bf16 multiply (2x mode) + ACT activation-accum
row-sum; 3 via DVE fused scalar_tensor_tensor). w = attn@v and the
projections run on TensorE in bf16 (single pass). Softmax uses
unnormalized exp (scores ~ N(0,1), no overflow in fp32) with 1/Z
folded into the w_row evacuation. Batch 1's DMAs are interleaved with
batch 0's v stream so its k chunks are resident when compute frees up.

Raw bass (not Tile): this toolchain's walrus build rejects >1 embedded
sync-wait per compute instruction, which Tile's scheduler emits; raw
bass uses standalone sequencer waits, with explicit semaphore ticks
precomputed in python (the *_seq tables below).

PSUM bank map (PE-W vs DVE/ACT-R hazards serialized via the sem chain;
bank granularity, not address: concurrent PE-write + DVE/ACT-read of
the SAME bank is a fatal HW error even at different addresses):
  bank 0    : qp_row [0:1,256:512], qp_col [:,0:2], Z [0:1,4:5],
              w fold [:,8:16], q fold [:,16:24]
  banks 1-2 : qt row / out row (disjoint lifetimes)
  banks 3-4 : qt broadcast [128, 1024]
  banks 5-6 : w accumulator row
"""

import contextlib

import numpy as np

try:
    import concourse.bass as bass  # noqa: F401
except ImportError:  # fallback if site path isn't preloaded
    import sys

    sys.path.insert(0, "/opt/trn_rl_repo")

B = 16
NCORES = 8
BPC = B // NCORES  # batches per core
NK = 4096
E = 1024
A = 256
NSUB = NK // 128   # 32 token subtiles of 128
CHUNK = 8          # subtiles per DMA chunk (2 MB in bf16)
NCHUNK = NSUB // CHUNK
KBUFS = 4
VBUFS = 4
NACT = 5           # tiles per chunk reduced on ACT (rest fused on DVE)
# scratch slot j is dedicated to tile position j of each chunk, so a
# chunk's multiplies only wait on the PREVIOUS chunk's reduces


def _build_nc():
    import concourse.bass as bass
    from concourse import mybir

    FP = mybir.dt.float32
    BF = mybir.dt.bfloat16
    AL = mybir.AluOpType
    AF = mybir.ActivationFunctionType

    nc = bass.Bass()
    q_d = nc.declare_dram_parameter("q", [BPC, E], BF, isOutput=False)
    k_d = nc.declare_dram_parameter("k", [BPC, NK, E], BF, isOutput=False)
    v_d = nc.declare_dram_parameter("v", [BPC, NK, E], BF, isOutput=False)
    wkq_d = nc.declare_dram_parameter("W_kq", [E, A], BF, isOutput=False)
    wkqT_d = nc.declare_dram_parameter("W_kqT", [A, E], BF, isOutput=False)
    bkq_d = nc.declare_dram_parameter("b_kq", [A], FP, isOutput=False)
    wv_d = nc.declare_dram_parameter("W_v", [E, E], BF, isOutput=False)
    bv_d = nc.declare_dram_parameter("b_v", [E], FP, isOutput=False)
    out_d = nc.declare_dram_parameter("out", [BPC, E], FP, isOutput=True)

    with contextlib.ExitStack() as st:
        def sb(name, shape, dt=FP):
            return st.enter_context(nc.sbuf_tensor(name, shape, dt))

        # ---- SBUF ----
        wkq_sb = sb("wkq_sb", [128, 8, A], BF)
        wkqT_sb = sb("wkqT_sb", [128, 2, E], BF)
        wv_sb = sb("wv_sb", [128, 8, E], BF)
        q_row = sb("q_row", [1, BPC * E], BF)
        qcol_sb = [sb(f"qcol_sb{b}", [128, 8], BF) for b in range(BPC)]
        bkq_row = sb("bkq_row", [1, A])
        bv_row = sb("bv_row", [1, E])
        ones_col = sb("ones_col", [128, 1])        # fp32 (Z rhs)
        ones_bf = sb("ones_bf", [1, 128], BF)      # bf16 (w fold rhs)
        kt = [sb(f"kt{i}", [128, CHUNK, E], BF) for i in range(KBUFS)]
        vt = [sb(f"vt{i}", [128, CHUNK, E], BF) for i in range(VBUFS)]
        scr = [sb(f"scr{i}", [128, E], BF) for i in range(NACT)]
        # per-batch smalls
        qpr_sb = [sb(f"qpr_sb{b}", [1, A], BF) for b in range(BPC)]
        qp_sb = [sb(f"qp_sb{b}", [128, 2], BF) for b in range(BPC)]
        qt_sb = [sb(f"qt_sb{b}", [1, E], BF) for b in range(BPC)]
        qtb_sb = [sb(f"qtb_sb{b}", [128, E], BF) for b in range(BPC)]
        smat = [sb(f"smat{b}", [128, NSUB]) for b in range(BPC)]
        pmat = [sb(f"pmat{b}", [128, NSUB], BF) for b in range(BPC)]
        psums = [sb(f"psums{b}", [128, 1]) for b in range(BPC)]
        invz = [sb(f"invz{b}", [1, 1]) for b in range(BPC)]
        w_row = [sb(f"w_row{b}", [1, E], BF) for b in range(BPC)]
        w_col = [sb(f"w_col{b}", [128, 8], BF) for b in range(BPC)]
        o_sb = [sb(f"o_sb{b}", [1, E]) for b in range(BPC)]

        # ---- PSUM (static bank map) ----
        ps_small = st.enter_context(nc.psum_tensor([128, 512], FP))   # bank 0
        ps_a = st.enter_context(nc.psum_tensor([128, 1024], FP))      # banks 1-2
        ps_b = st.enter_context(nc.psum_tensor([128, 1024], FP))      # banks 3-4
        ps_w = st.enter_context(nc.psum_tensor([128, 1024], FP))      # banks 5-6

        # ---- semaphores ----
        sW = st.enter_context(nc.semaphore("sW"))      # wkq+wkqT+q+bkq -> 64
        sWV = st.enter_context(nc.semaphore("sWV"))    # wv -> 16
        sBV = st.enter_context(nc.semaphore("sBV"))    # bv -> 16
        sK = [st.enter_context(nc.semaphore(f"sK{i}")) for i in range(KBUFS)]
        sV = [st.enter_context(nc.semaphore(f"sV{i}")) for i in range(VBUFS)]
        sOUT = st.enter_context(nc.semaphore("sOUT"))
        sPE = st.enter_context(nc.semaphore("sPE"))
        sDVE = st.enter_context(nc.semaphore("sDVE"))
        sACT = st.enter_context(nc.semaphore("sACT"))

        blk = st.enter_context(nc.Block())

        # ---------- event tick registry ----------
        def ticks(seq):
            return {ev: i + 1 for i, ev in enumerate(seq)}

        pe_seq = ["QF0", "QPROW0", "QF1", "QPF0", "QT0", "QTB0",
                  "QPROW1", "QPF1", "QT1", "QTB1"]
        pe_seq += ["Z0"] + [f"W0C{c}" for c in range(NCHUNK)]
        pe_seq += ["Z1", "W1C0", "FOLD0", "W1C1", "PROJ0", "W1C2", "W1C3",
                   "FOLD1", "PROJ1"]
        PE = ticks(pe_seq)

        def mult_ev(b, c, j):
            # DVE inc for tile j of chunk (b, c): mult (j < NACT) or fused stt
            return f"MUL{b}_{c}_{j}"

        def red_ev(b, c, j):
            return f"RED{b}_{c}_{j}"

        dve_seq = ["MS1", "MS2", "QCOL0", "QPRSB0", "QCOL1", "QPSB0",
                   "QTBSB0"]
        dve_seq += [mult_ev(0, 0, j) for j in range(CHUNK)]
        dve_seq += ["QPRSB1"]
        dve_seq += [mult_ev(0, 1, j) for j in range(CHUNK)]
        dve_seq += ["QPSB1"]
        dve_seq += [mult_ev(0, 2, j) for j in range(CHUNK)]
        dve_seq += ["QTBSB1"]
        dve_seq += [mult_ev(0, 3, j) for j in range(CHUNK)]
        dve_seq += [mult_ev(1, 0, j) for j in range(CHUNK)]
        dve_seq += ["INVZ0"]
        dve_seq += [mult_ev(1, 1, j) for j in range(CHUNK)]
        dve_seq += [mult_ev(1, 2, j) for j in range(CHUNK)]
        dve_seq += [mult_ev(1, 3, j) for j in range(CHUNK)]
        dve_seq += ["WCOL0", "OSB0", "INVZ1", "WCOL1", "OSB1"]
        DVE = ticks(dve_seq)

        act_seq = ["QTSB0"]
        act_seq += [red_ev(0, 0, j) for j in range(NACT)]
        act_seq += ["QTSB1"]
        for c in range(1, NCHUNK):
            act_seq += [red_ev(0, c, j) for j in range(NACT)]
        act_seq += ["EXP0"]
        for c in range(NCHUNK):
            act_seq += [red_ev(1, c, j) for j in range(NACT)]
        act_seq += ["EXP1", "WROW0", "WROW1"]
        ACT = ticks(act_seq)

        # ---------- SYNC: all DMAs ----------
        @blk.sync
        def _(sync):
            sync.dma_start(
                out=wkq_sb[:], in_=wkq_d[:].rearrange("(dc p) a -> p dc a", p=128)
            ).then_inc(sW, 16)
            sync.dma_start(
                out=wkqT_sb[:], in_=wkqT_d[:].rearrange("(ac p) d -> p ac d", p=128)
            ).then_inc(sW, 16)
            sync.dma_start(
                out=q_row[:], in_=q_d[:].rearrange("b e -> (b e)")[None, :]
            ).then_inc(sW, 16)
            sync.dma_start(out=bkq_row[:], in_=bkq_d[:][None, :]).then_inc(sW, 16)
            sync.dma_start(out=bv_row[:], in_=bv_d[:][None, :]).then_inc(sBV, 16)

            def kdma(b, c):
                g = b * NCHUNK + c
                if g >= KBUFS:
                    gp = g - KBUFS
                    bp, cp = divmod(gp, NCHUNK)
                    sync.wait_ge(sDVE, DVE[mult_ev(bp, cp, CHUNK - 1)])
                k_b = k_d[:][b].rearrange("(p s) d -> p s d", p=128)
                sync.dma_start(
                    out=kt[g % KBUFS][:],
                    in_=k_b[:, c * CHUNK:(c + 1) * CHUNK, :],
                ).then_inc(sK[g % KBUFS], 16)

            def vdma(b, c):
                g = b * NCHUNK + c
                if g >= VBUFS:
                    gp = g - VBUFS
                    bp, cp = divmod(gp, NCHUNK)
                    sync.wait_ge(sPE, PE[f"W{bp}C{cp}"])
                v_b = v_d[:][b].rearrange("(p s) d -> p s d", p=128)
                sync.dma_start(
                    out=vt[g % VBUFS][:],
                    in_=v_b[:, c * CHUNK:(c + 1) * CHUNK, :],
                ).then_inc(sV[g % VBUFS], 16)

            # k1 interleaved with v0 so it streams during k0/v0 compute;
            # PE-gated issues (v0c3, k1c3) pushed late to avoid head-of-line
            for c in range(NCHUNK):
                kdma(0, c)
            sync.dma_start(
                out=wv_sb[:], in_=wv_d[:].rearrange("(dc p) e -> p dc e", p=128)
            ).then_inc(sWV, 16)
            vdma(0, 0)
            vdma(0, 1)
            vdma(0, 2)
            kdma(1, 0)
            kdma(1, 1)
            vdma(0, 3)
            kdma(1, 2)
            kdma(1, 3)
            for c in range(NCHUNK):
                vdma(1, c)

            for b in range(BPC):
                sync.wait_ge(sDVE, DVE[f"OSB{b}"])
                sync.dma_start(out=out_d[:][b:b + 1, :], in_=o_sb[b][:]).then_inc(
                    sOUT, 16)
            sync.wait_ge(sOUT, BPC * 16)

        # ---------- PE ----------
        @blk.tensor
        def _(tensor):
            tensor.wait_ge(sW, 64)
            tensor.wait_ge(sDVE, DVE["MS2"])  # ones tiles ready
            def qfold(b):
                if b > 0:
                    # bank-0 serialization: latest possible concurrent reader
                    tensor.wait_ge(sDVE, DVE[f"QPRSB{b - 1}"])
                for dc in range(8):
                    mm = tensor.matmul(
                        out=ps_small[:, 16 + dc:17 + dc],
                        lhsT=q_row[0:1, b * E + dc * 128:b * E + (dc + 1) * 128],
                        rhs=ones_bf[0:1, 0:1],
                        start=True, stop=True,
                    )
                mm.then_inc(sPE, 1)                      # QF{b}

            def qprow(b):
                tensor.wait_ge(sDVE, DVE[f"QCOL{b}"])
                if b > 0:
                    # bank-0 safety: prior batch's bank-0 reads done
                    tensor.wait_ge(sDVE, DVE[f"QPSB{b - 1}"])
                for dc in range(8):
                    mm = tensor.matmul(
                        out=ps_small[0:1, 256:256 + A],
                        lhsT=qcol_sb[b][:, dc:dc + 1],
                        rhs=wkq_sb[:, dc, :],
                        start=(dc == 0), stop=(dc == 7),
                    )
                mm.then_inc(sPE, 1)                      # QPROW{b}

            def qpf(b):
                tensor.wait_ge(sDVE, DVE[f"QPRSB{b}"])
                if b == 0:
                    # bank-0 serialization vs QCOL1's read
                    tensor.wait_ge(sDVE, DVE["QCOL1"])
                for c2 in range(2):
                    mm = tensor.matmul(
                        out=ps_small[:, c2:c2 + 1],
                        lhsT=qpr_sb[b][0:1, c2 * 128:(c2 + 1) * 128],
                        rhs=ones_bf[0:1, 0:1],
                        start=True, stop=True,
                    )
                mm.then_inc(sPE, 1)                      # QPF{b}

            def qt_mm(b):
                tensor.wait_ge(sDVE, DVE[f"QPSB{b}"])
                if b > 0:
                    tensor.wait_ge(sACT, ACT[f"QTSB{b - 1}"])
                for ac in range(2):
                    for nh in range(2):
                        mm = tensor.matmul(
                            out=ps_a[0:1, nh * 512:(nh + 1) * 512],
                            lhsT=qp_sb[b][:, ac:ac + 1],
                            rhs=wkqT_sb[:, ac, nh * 512:(nh + 1) * 512],
                            start=(ac == 0), stop=(ac == 1),
                        )
                mm.then_inc(sPE, 1)                      # QT{b}

            def qtb_mm(b):
                tensor.wait_ge(sACT, ACT[f"QTSB{b}"])
                for nh in range(2):
                    mm = tensor.matmul(
                        out=ps_b[:, nh * 512:(nh + 1) * 512],
                        lhsT=ones_bf[:],
                        rhs=qt_sb[b][0:1, nh * 512:(nh + 1) * 512],
                        start=True, stop=True,
                    )
                mm.then_inc(sPE, 1)                      # QTB{b}

            qfold(0)
            qprow(0)
            qfold(1)
            qpf(0)
            qt_mm(0)
            qtb_mm(0)
            qprow(1)
            qpf(1)
            qt_mm(1)
            qtb_mm(1)

            def z_mm(b):
                tensor.wait_ge(sACT, ACT[f"EXP{b}"])
                tensor.matmul(
                    out=ps_small[0:1, 4:5], lhsT=psums[b][:], rhs=ones_col[:],
                    start=True, stop=True,
                ).then_inc(sPE, 1)                       # Z{b}

            def w_chunk(b, c):
                g = b * NCHUNK + c
                tensor.wait_ge(sV[g % VBUFS], (g // VBUFS + 1) * 16)
                if b == 1 and c == 0:
                    # ps_w WAR: batch-0 w_row evacuation must complete
                    tensor.wait_ge(sACT, ACT["WROW0"])
                for j in range(CHUNK):
                    t = c * CHUNK + j
                    for nh in range(2):
                        mm = tensor.matmul(
                            out=ps_w[0:1, nh * 512:(nh + 1) * 512],
                            lhsT=pmat[b][:, t:t + 1],
                            rhs=vt[g % VBUFS][:, j, nh * 512:(nh + 1) * 512],
                            start=(t == 0), stop=(t == NSUB - 1),
                        )
                mm.then_inc(sPE, 1)                      # W{b}C{c}

            def fold_mm(b):
                tensor.wait_ge(sACT, ACT[f"WROW{b}"])
                for dc in range(8):
                    mm = tensor.matmul(
                        out=ps_small[:, 8 + dc:9 + dc],
                        lhsT=w_row[b][0:1, dc * 128:(dc + 1) * 128],
                        rhs=ones_bf[0:1, 0:1],
                        start=True, stop=True,
                    )
                mm.then_inc(sPE, 1)                      # FOLD{b}

            def proj_mm(b):
                tensor.wait_ge(sDVE, DVE[f"WCOL{b}"])
                if b == 0:
                    tensor.wait_ge(sWV, 16)
                    tensor.wait_ge(sACT, ACT["QTSB1"])   # ps_a overwrite guard
                for dc in range(8):
                    for nh in range(2):
                        mm = tensor.matmul(
                            out=ps_a[0:1, nh * 512:(nh + 1) * 512],
                            lhsT=w_col[b][:, dc:dc + 1],
                            rhs=wv_sb[:, dc, nh * 512:(nh + 1) * 512],
                            start=(dc == 0), stop=(dc == 7),
                        )
                mm.then_inc(sPE, 1)                      # PROJ{b}

            # batch-0 tail (fold0/proj0) threads through w1's DMA-pacing gaps
            z_mm(0)
            for c in range(NCHUNK):
                w_chunk(0, c)
            z_mm(1)
            w_chunk(1, 0)
            fold_mm(0)
            w_chunk(1, 1)
            proj_mm(0)
            w_chunk(1, 2)
            w_chunk(1, 3)
            fold_mm(1)
            proj_mm(1)

        # ---------- DVE ----------
        @blk.vector
        def _(vector):
            vector.memset(ones_col[:], 1.0).then_inc(sDVE, 1)
            vector.memset(ones_bf[:], 1.0).then_inc(sDVE, 1)

            def qcol(b):
                vector.wait_ge(sPE, PE[f"QF{b}"])
                vector.tensor_copy(out=qcol_sb[b][:], in_=ps_small[:, 16:24]) \
                    .then_inc(sDVE, 1)                                # QCOL{b}

            def small_chain(b, step):
                if step == 0:
                    if b == 0:
                        vector.wait_ge(sW, 64)
                    vector.wait_ge(sPE, PE[f"QPROW{b}"])
                    vector.tensor_add(qpr_sb[b][:], ps_small[0:1, 256:256 + A],
                                      bkq_row[:]).then_inc(sDVE, 1)   # QPRSB{b}
                elif step == 1:
                    vector.wait_ge(sPE, PE[f"QPF{b}"])
                    vector.tensor_copy(out=qp_sb[b][:], in_=ps_small[:, 0:2]) \
                        .then_inc(sDVE, 1)                            # QPSB{b}
                else:
                    vector.wait_ge(sPE, PE[f"QTB{b}"])
                    vector.tensor_copy(out=qtb_sb[b][:], in_=ps_b[:]) \
                        .then_inc(sDVE, 1)                            # QTBSB{b}

            def mult_chunk(b, c):
                g = b * NCHUNK + c
                vector.wait_ge(sK[g % KBUFS], (g // KBUFS + 1) * 16)
                if c == 0:
                    # self-wait: qtb_sb copy completion before reads
                    vector.wait_ge(sDVE, DVE[f"QTBSB{b}"])
                for j in range(CHUNK):
                    t = c * CHUNK + j
                    if j < NACT:
                        if g >= 1:
                            # scratch slot j: previous chunk's reduce done
                            bp, cp = divmod(g - 1, NCHUNK)
                            vector.wait_ge(sACT, ACT[red_ev(bp, cp, j)])
                        vector.tensor_mul(
                            scr[j][:], kt[g % KBUFS][:, j, :], qtb_sb[b][:]
                        ).then_inc(sDVE, 1)               # MUL{b}_{c}_{j}
                    else:
                        # fused dot product on DVE, in-place on the k slice:
                        # out=(k*1)*qt, accum_out=row sum
                        vector.scalar_tensor_tensor(
                            out=kt[g % KBUFS][:, j, :],
                            in0=kt[g % KBUFS][:, j, :], scalar=1.0,
                            in1=qtb_sb[b][:],
                            op0=AL.mult, op1=AL.mult,
                            accum_out=smat[b][:, t:t + 1],
                        ).then_inc(sDVE, 1)               # MUL{b}_{c}_{j}

            def tail(b, step):
                if step == 0:
                    vector.wait_ge(sPE, PE[f"Z{b}"])
                    vector.reciprocal(invz[b][:], ps_small[0:1, 4:5]) \
                        .then_inc(sDVE, 1)                            # INVZ{b}
                elif step == 1:
                    vector.wait_ge(sPE, PE[f"FOLD{b}"])
                    vector.tensor_copy(out=w_col[b][:], in_=ps_small[:, 8:16]) \
                        .then_inc(sDVE, 1)                            # WCOL{b}
                else:
                    vector.wait_ge(sPE, PE[f"PROJ{b}"])
                    if b == 0:
                        vector.wait_ge(sBV, 16)
                    vector.tensor_add(o_sb[b][:], ps_a[0:1, :], bv_row[:]) \
                        .then_inc(sDVE, 1)                            # OSB{b}

            qcol(0)
            small_chain(0, 0)
            qcol(1)
            small_chain(0, 1)
            small_chain(0, 2)
            mult_chunk(0, 0)
            small_chain(1, 0)
            mult_chunk(0, 1)
            small_chain(1, 1)
            mult_chunk(0, 2)
            small_chain(1, 2)
            mult_chunk(0, 3)
            mult_chunk(1, 0)
            tail(0, 0)          # INVZ0
            mult_chunk(1, 1)
            mult_chunk(1, 2)
            mult_chunk(1, 3)
            tail(0, 1)          # WCOL0
            tail(0, 2)          # OSB0
            tail(1, 0)
            tail(1, 1)
            tail(1, 2)

        # ---------- ACT (scalar) ----------
        @blk.scalar
        def _(scalar):
            def qtsb(b):
                scalar.wait_ge(sPE, PE[f"QT{b}"])
                scalar.mul(qt_sb[b][:], ps_a[0:1, :], 1.0 / 16.0) \
                    .then_inc(sACT, 1)                                # QTSB{b}

            def red_chunk(b, c):
                for j in range(NACT):
                    t = c * CHUNK + j
                    scalar.wait_ge(sDVE, DVE[mult_ev(b, c, j)])
                    scalar.activation(
                        out=scr[j][:], in_=scr[j][:], func=AF.Copy,
                        accum_out=smat[b][:, t:t + 1],
                    ).then_inc(sACT, 1)                   # RED{b}_{c}_{j}

            def expb(b):
                # smat writers: ACT reduces (self-order) + DVE fused stts
                scalar.wait_ge(sACT, ACT[red_ev(b, NCHUNK - 1, NACT - 1)])
                scalar.wait_ge(sDVE, DVE[mult_ev(b, NCHUNK - 1, CHUNK - 1)])
                scalar.activation(
                    out=pmat[b][:], in_=smat[b][:], func=AF.Exp,
                    accum_out=psums[b][:],
                ).then_inc(sACT, 1)                                   # EXP{b}

            def wrow(b):
                scalar.wait_ge(sPE, PE[f"W{b}C{NCHUNK - 1}"])
                scalar.wait_ge(sDVE, DVE[f"INVZ{b}"])
                scalar.activation(
                    out=w_row[b][:], in_=ps_w[0:1, :], func=AF.Copy,
                    bias=0.0, scale=invz[b][0:1, 0:1],
                ).then_inc(sACT, 1)                                   # WROW{b}

            qtsb(0)
            red_chunk(0, 0)
            qtsb(1)
            for c in range(1, NCHUNK):
                red_chunk(0, c)
            expb(0)
            for c in range(NCHUNK):
                red_chunk(1, c)
            expb(1)
            wrow(0)
            wrow(1)

    return nc


_NC_CACHE = None


def get_nc():
    global _NC_CACHE
    if _NC_CACHE is None:
        _NC_CACHE = _build_nc()
    return _NC_CACHE


def make_in_maps(q, k, v, W_kq, b_kq, W_v, b_v):
    """Shard full inputs over 8 cores: batch-parallel, weights replicated.
    k, v, W_v are cast to bfloat16 on the host (compute dtype of the
    streaming contractions)."""
    import ml_dtypes

    bf16 = ml_dtypes.bfloat16
    q = np.ascontiguousarray(
        np.asarray(q, dtype=np.float32).reshape(B, E).astype(bf16))
    k = np.ascontiguousarray(np.asarray(k, dtype=np.float32).astype(bf16))
    v = np.ascontiguousarray(np.asarray(v, dtype=np.float32).astype(bf16))
    W_kq32 = np.asarray(W_kq, dtype=np.float32)
    W_kq = np.ascontiguousarray(W_kq32.astype(bf16))
    W_kqT = np.ascontiguousarray(W_kq32.T.astype(bf16))
    b_kq = np.ascontiguousarray(np.asarray(b_kq, dtype=np.float32))
    W_v = np.ascontiguousarray(np.asarray(W_v, dtype=np.float32).astype(bf16))
    b_v = np.ascontiguousarray(np.asarray(b_v, dtype=np.float32))
    in_maps = []
    for i in range(NCORES):
        lo, hi = i * BPC, (i + 1) * BPC
        in_maps.append({
            "q": q[lo:hi],
            "k": k[lo:hi],
            "v": v[lo:hi],
            "W_kq": W_kq,
            "W_kqT": W_kqT,
            "b_kq": b_kq,
            "W_v": W_v,
            "b_v": b_v,
        })
    return in_maps


def kernel(q, k, v, W_kq, b_kq, W_v, b_v):
    from concourse.bass_utils import run_bass_kernel_spmd

    nc = get_nc()
    in_maps = make_in_maps(q, k, v, W_kq, b_kq, W_v, b_v)
    res = run_bass_kernel_spmd(nc, in_maps, core_ids=list(range(NCORES)))
    out = np.concatenate([res.results[i]["out"] for i in range(NCORES)], axis=0)
    return np.ascontiguousarray(out.astype(np.float32))


# revision 53
# speedup vs baseline: 1.0970x; 1.0111x over previous
"""Distributed attention kernel for Trainium2 (8 NeuronCores, SPMD).

Problem: B=16 batches of single-query attention over NK=4096 keys,
EMBED=1024, ATTN=256, with a shared kq projection and a v projection.

Math restructuring (exact up to float reassociation):
  - scores = (q@W_kq + b_kq) @ (k@W_kq + b_kq)^T / 16
           = k @ qt + const            where qt = W_kq @ (W_kq^T q + b_kq) / 16
    (the constant offsets every score equally -> softmax invariant, dropped)
  - out = softmax(scores) @ (v@W_v + b_v)
        = (attn @ v) @ W_v + b_v       (attn sums to 1)
This removes the O(NK*E*E) v-projection and O(NK*E*A) k-projection
entirely; the kernel is HBM-bandwidth bound streaming k and v once.

Sharding: data-parallel over batch, 2 batches per core. k, v, W_v are
cast to bf16 on the host (compute dtype of the streaming contractions,
halves HBM traffic); W_kq is additionally passed pre-transposed.

Token layout is p-major ("(p s) d"): partition p holds NSUB consecutive
token rows, so each chunk DMA is one 16KB-contiguous run per partition
(128 descriptors, not 1024). k and v use the same permutation, and
softmax is globally permutation-invariant, so results are unchanged.

s = k.qt is load-balanced across engines per 128-token tile (per chunk
of 8 tiles: 5 via DVE bf16 multiply (2x mode) + ACT activation-accum
row-sum; 3 via DVE fused scalar_tensor_tensor). w = attn@v and the
projections run on TensorE in bf16 (single pass). Softmax uses
unnormalized exp (scores ~ N(0,1), no overflow in fp32) with 1/Z
folded into the w_row evacuation. Batch 1's DMAs are interleaved with
batch 0's v stream so its k chunks are resident when compute frees up.

Raw bass (not Tile): this toolchain's walrus build rejects >1 embedded
sync-wait per compute instruction, which Tile's scheduler emits; raw
bass uses standalone sequencer waits, with explicit semaphore ticks
precomputed in python (the *_seq tables below).

PSUM bank map (PE-W vs DVE/ACT-R hazards serialized via the sem chain;
bank granularity, not address: concurrent PE-write + DVE/ACT-read of
the SAME bank is a fatal HW error even at different addresses):
  bank 0    : qp_row [0:1,256:512], qp_col [:,0:2], Z [0:1,4:5],
              w fold [:,8:16], q fold [:,16:24]
  banks 1-2 : qt row / out row (disjoint lifetimes)
  banks 3-4 : qt broadcast [128, 1024]
  banks 5-6 : w accumulator row
"""

import contextlib

import numpy as np

try:
    import concourse.bass as bass  # noqa: F401
except ImportError:  # fallback if site path isn't preloaded
    import sys

    sys.path.insert(0, "/opt/trn_rl_repo")

B = 16
NCORES = 8
BPC = B // NCORES  # batches per core
NK = 4096
E = 1024
A = 256
NSUB = NK // 128   # 32 token subtiles of 128
CHUNK = 8          # subtiles per DMA chunk (2 MB in bf16)
NCHUNK = NSUB // CHUNK
KBUFS = 4
VBUFS = 4
NACT = 5           # tiles per chunk reduced on ACT (rest fused on DVE)
# scratch slot j is dedicated to tile position j of each chunk, so a
# chunk's multiplies only wait on the PREVIOUS chunk's reduces


def _build_nc():
    import concourse.bass as bass
    from concourse import mybir

    FP = mybir.dt.float32
    BF = mybir.dt.bfloat16
    AL = mybir.AluOpType
    AF = mybir.ActivationFunctionType

    nc = bass.Bass()
    q_d = nc.declare_dram_parameter("q", [BPC, E], BF, isOutput=False)
    k_d = nc.declare_dram_parameter("k", [BPC, NK, E], BF, isOutput=False)
    v_d = nc.declare_dram_parameter("v", [BPC, NK, E], BF, isOutput=False)
    wkq_d = nc.declare_dram_parameter("W_kq", [E, A], BF, isOutput=False)
    wkqT_d = nc.declare_dram_parameter("W_kqT", [A, E], BF, isOutput=False)
    bkq_d = nc.declare_dram_parameter("b_kq", [A], FP, isOutput=False)
    wv_d = nc.declare_dram_parameter("W_v", [E, E], BF, isOutput=False)
    bv_d = nc.declare_dram_parameter("b_v", [E], FP, isOutput=False)
    out_d = nc.declare_dram_parameter("out", [BPC, E], FP, isOutput=True)

    with contextlib.ExitStack() as st:
        def sb(name, shape, dt=FP):
            return st.enter_context(nc.sbuf_tensor(name, shape, dt))

        # ---- SBUF ----
        wkq_sb = sb("wkq_sb", [128, 8, A], BF)
        wkqT_sb = sb("wkqT_sb", [128, 2, E], BF)
        wv_sb = sb("wv_sb", [128, 8, E], BF)
        q_row = sb("q_row", [1, BPC * E], BF)
        qcol_sb = [sb(f"qcol_sb{b}", [128, 8], BF) for b in range(BPC)]
        bkq_row = sb("bkq_row", [1, A])
        bv_row = sb("bv_row", [1, E])
        ones_col = sb("ones_col", [128, 1])        # fp32 (Z rhs)
        ones_bf = sb("ones_bf", [1, 128], BF)      # bf16 (w fold rhs)
        kt = [sb(f"kt{i}", [128, CHUNK, E], BF) for i in range(KBUFS)]
        vt = [sb(f"vt{i}", [128, CHUNK, E], BF) for i in range(VBUFS)]
        scr = [sb(f"scr{i}", [128, E], BF) for i in range(NACT)]
        # per-batch smalls
        qpr_sb = [sb(f"qpr_sb{b}", [1, A], BF) for b in range(BPC)]
        qp_sb = [sb(f"qp_sb{b}", [128, 2], BF) for b in range(BPC)]
        qt_sb = [sb(f"qt_sb{b}", [1, E], BF) for b in range(BPC)]
        qtb_sb = [sb(f"qtb_sb{b}", [128, E], BF) for b in range(BPC)]
        smat = [sb(f"smat{b}", [128, NSUB]) for b in range(BPC)]
        pmat = [sb(f"pmat{b}", [128, NSUB], BF) for b in range(BPC)]
        zpart = [sb(f"zpart{b}", [128, NCHUNK]) for b in range(BPC)]
        zredc = [sb(f"zredc{b}", [128, 1]) for b in range(BPC)]
        invz = [sb(f"invz{b}", [1, 1]) for b in range(BPC)]
        w_row = [sb(f"w_row{b}", [1, E], BF) for b in range(BPC)]
        w_col = [sb(f"w_col{b}", [128, 8], BF) for b in range(BPC)]
        o_sb = [sb(f"o_sb{b}", [1, E]) for b in range(BPC)]

        # ---- PSUM (static bank map) ----
        ps_small = st.enter_context(nc.psum_tensor([128, 512], FP))   # bank 0
        ps_a = st.enter_context(nc.psum_tensor([128, 1024], FP))      # banks 1-2
        ps_b = st.enter_context(nc.psum_tensor([128, 1024], FP))      # banks 3-4
        ps_w = st.enter_context(nc.psum_tensor([128, 1024], FP))      # banks 5-6

        # ---- semaphores ----
        sW = st.enter_context(nc.semaphore("sW"))      # wkq+wkqT+q+bkq -> 64
        sWV = st.enter_context(nc.semaphore("sWV"))    # wv -> 16
        sBV = st.enter_context(nc.semaphore("sBV"))    # bv -> 16
        sK = [st.enter_context(nc.semaphore(f"sK{i}")) for i in range(KBUFS)]
        sV = [st.enter_context(nc.semaphore(f"sV{i}")) for i in range(VBUFS)]
        sOUT = st.enter_context(nc.semaphore("sOUT"))
        sPE = st.enter_context(nc.semaphore("sPE"))
        sDVE = st.enter_context(nc.semaphore("sDVE"))
        sACT = st.enter_context(nc.semaphore("sACT"))

        blk = st.enter_context(nc.Block())

        # ---------- event tick registry ----------
        def ticks(seq):
            return {ev: i + 1 for i, ev in enumerate(seq)}

        pe_seq = ["QF0", "QPROW0", "QF1", "QPF0", "QT0", "QTB0",
                  "QPROW1", "QPF1", "QT1", "QTB1"]
        pe_seq += [f"W0C{c}" for c in range(NCHUNK)] + ["Z0"]
        pe_seq += ["W1C0", "FOLD0", "W1C1", "PROJ0", "W1C2", "W1C3",
                   "Z1", "FOLD1", "PROJ1"]
        PE = ticks(pe_seq)

        def mult_ev(b, c, j):
            # DVE inc for tile j of chunk (b, c): mult (j < NACT) or fused stt
            return f"MUL{b}_{c}_{j}"

        def red_ev(b, c, j):
            return f"RED{b}_{c}_{j}"

        dve_seq = ["MS1", "MS2", "QCOL0", "QPRSB0", "QCOL1", "QPSB0",
                   "QTBSB0"]
        dve_seq += [mult_ev(0, 0, j) for j in range(CHUNK)]
        dve_seq += ["QPRSB1"]
        dve_seq += [mult_ev(0, 1, j) for j in range(CHUNK)]
        dve_seq += ["QPSB1"]
        dve_seq += [mult_ev(0, 2, j) for j in range(CHUNK)]
        dve_seq += ["QTBSB1"]
        dve_seq += [mult_ev(0, 3, j) for j in range(CHUNK)]
        dve_seq += [mult_ev(1, 0, j) for j in range(CHUNK)]
        dve_seq += ["ZRED0"]
        dve_seq += [mult_ev(1, 1, j) for j in range(CHUNK)]
        dve_seq += ["INVZ0"]
        dve_seq += [mult_ev(1, 2, j) for j in range(CHUNK)]
        dve_seq += ["WCOL0"]
        dve_seq += [mult_ev(1, 3, j) for j in range(CHUNK)]
        dve_seq += ["OSB0", "ZRED1", "INVZ1", "WCOL1", "OSB1"]
        DVE = ticks(dve_seq)

        act_seq = ["QTSB0"]
        act_seq += [red_ev(0, 0, j) for j in range(NACT)] + ["EXPC0_0", "QTSB1"]
        for c in range(1, NCHUNK):
            act_seq += [red_ev(0, c, j) for j in range(NACT)] + [f"EXPC0_{c}"]
        act_seq += [red_ev(1, 0, j) for j in range(NACT)] + ["EXPC1_0"]
        act_seq += [red_ev(1, 1, j) for j in range(NACT)] + ["EXPC1_1", "WROW0"]
        act_seq += [red_ev(1, 2, j) for j in range(NACT)] + ["EXPC1_2"]
        act_seq += [red_ev(1, 3, j) for j in range(NACT)] + ["EXPC1_3", "WROW1"]
        ACT = ticks(act_seq)

        # ---------- SYNC: all DMAs ----------
        @blk.sync
        def _(sync):
            sync.dma_start(
                out=wkq_sb[:], in_=wkq_d[:].rearrange("(dc p) a -> p dc a", p=128)
            ).then_inc(sW, 16)
            sync.dma_start(
                out=wkqT_sb[:], in_=wkqT_d[:].rearrange("(ac p) d -> p ac d", p=128)
            ).then_inc(sW, 16)
            sync.dma_start(
                out=q_row[:], in_=q_d[:].rearrange("b e -> (b e)")[None, :]
            ).then_inc(sW, 16)
            sync.dma_start(out=bkq_row[:], in_=bkq_d[:][None, :]).then_inc(sW, 16)
            sync.dma_start(out=bv_row[:], in_=bv_d[:][None, :]).then_inc(sBV, 16)

            def kdma(b, c):
                g = b * NCHUNK + c
                if g >= KBUFS:
                    gp = g - KBUFS
                    bp, cp = divmod(gp, NCHUNK)
                    sync.wait_ge(sDVE, DVE[mult_ev(bp, cp, CHUNK - 1)])
                k_b = k_d[:][b].rearrange("(p s) d -> p s d", p=128)
                sync.dma_start(
                    out=kt[g % KBUFS][:],
                    in_=k_b[:, c * CHUNK:(c + 1) * CHUNK, :],
                ).then_inc(sK[g % KBUFS], 16)

            def vdma(b, c):
                g = b * NCHUNK + c
                if g >= VBUFS:
                    gp = g - VBUFS
                    bp, cp = divmod(gp, NCHUNK)
                    sync.wait_ge(sPE, PE[f"W{bp}C{cp}"])
                v_b = v_d[:][b].rearrange("(p s) d -> p s d", p=128)
                sync.dma_start(
                    out=vt[g % VBUFS][:],
                    in_=v_b[:, c * CHUNK:(c + 1) * CHUNK, :],
                ).then_inc(sV[g % VBUFS], 16)

            # online softmax: k and v chunks interleave per batch (w-MMs
            # consume each v chunk right after its k chunk's exp)
            kdma(0, 0)
            kdma(0, 1)
            vdma(0, 0)
            kdma(0, 2)
            vdma(0, 1)
            kdma(0, 3)
            vdma(0, 2)
            vdma(0, 3)
            sync.dma_start(
                out=wv_sb[:], in_=wv_d[:].rearrange("(dc p) e -> p dc e", p=128)
            ).then_inc(sWV, 16)
            kdma(1, 0)
            kdma(1, 1)
            vdma(1, 0)
            kdma(1, 2)
            vdma(1, 1)
            kdma(1, 3)
            vdma(1, 2)
            vdma(1, 3)

            for b in range(BPC):
                sync.wait_ge(sDVE, DVE[f"OSB{b}"])
                sync.dma_start(out=out_d[:][b:b + 1, :], in_=o_sb[b][:]).then_inc(
                    sOUT, 16)
            sync.wait_ge(sOUT, BPC * 16)

        # ---------- PE ----------
        @blk.tensor
        def _(tensor):
            tensor.wait_ge(sW, 64)
            tensor.wait_ge(sDVE, DVE["MS2"])  # ones tiles ready
            def qfold(b):
                if b > 0:
                    # bank-0 serialization: latest possible concurrent reader
                    tensor.wait_ge(sDVE, DVE[f"QPRSB{b - 1}"])
                for dc in range(8):
                    mm = tensor.matmul(
                        out=ps_small[:, 16 + dc:17 + dc],
                        lhsT=q_row[0:1, b * E + dc * 128:b * E + (dc + 1) * 128],
                        rhs=ones_bf[0:1, 0:1],
                        start=True, stop=True,
                    )
                mm.then_inc(sPE, 1)                      # QF{b}

            def qprow(b):
                tensor.wait_ge(sDVE, DVE[f"QCOL{b}"])
                if b > 0:
                    # bank-0 safety: prior batch's bank-0 reads done
                    tensor.wait_ge(sDVE, DVE[f"QPSB{b - 1}"])
                for dc in range(8):
                    mm = tensor.matmul(
                        out=ps_small[0:1, 256:256 + A],
                        lhsT=qcol_sb[b][:, dc:dc + 1],
                        rhs=wkq_sb[:, dc, :],
                        start=(dc == 0), stop=(dc == 7),
                    )
                mm.then_inc(sPE, 1)                      # QPROW{b}

            def qpf(b):
                tensor.wait_ge(sDVE, DVE[f"QPRSB{b}"])
                if b == 0:
                    # bank-0 serialization vs QCOL1's read
                    tensor.wait_ge(sDVE, DVE["QCOL1"])
                for c2 in range(2):
                    mm = tensor.matmul(
                        out=ps_small[:, c2:c2 + 1],
                        lhsT=qpr_sb[b][0:1, c2 * 128:(c2 + 1) * 128],
                        rhs=ones_bf[0:1, 0:1],
                        start=True, stop=True,
                    )
                mm.then_inc(sPE, 1)                      # QPF{b}

            def qt_mm(b):
                tensor.wait_ge(sDVE, DVE[f"QPSB{b}"])
                if b > 0:
                    tensor.wait_ge(sACT, ACT[f"QTSB{b - 1}"])
                for ac in range(2):
                    for nh in range(2):
                        mm = tensor.matmul(
                            out=ps_a[0:1, nh * 512:(nh + 1) * 512],
                            lhsT=qp_sb[b][:, ac:ac + 1],
                            rhs=wkqT_sb[:, ac, nh * 512:(nh + 1) * 512],
                            start=(ac == 0), stop=(ac == 1),
                        )
                mm.then_inc(sPE, 1)                      # QT{b}

            def qtb_mm(b):
                tensor.wait_ge(sACT, ACT[f"QTSB{b}"])
                for nh in range(2):
                    mm = tensor.matmul(
                        out=ps_b[:, nh * 512:(nh + 1) * 512],
                        lhsT=ones_bf[:],
                        rhs=qt_sb[b][0:1, nh * 512:(nh + 1) * 512],
                        start=True, stop=True,
                    )
                mm.then_inc(sPE, 1)                      # QTB{b}

            qfold(0)
            qprow(0)
            qfold(1)
            qpf(0)
            qt_mm(0)
            qtb_mm(0)
            qprow(1)
            qpf(1)
            qt_mm(1)
            qtb_mm(1)

            def z_mm(b):
                tensor.wait_ge(sDVE, DVE[f"ZRED{b}"])
                tensor.matmul(
                    out=ps_small[0:1, 4:5], lhsT=zredc[b][:], rhs=ones_col[:],
                    start=True, stop=True,
                ).then_inc(sPE, 1)                       # Z{b}

            def w_chunk(b, c):
                # online accumulation: batch b's w sums into its own psum
                # (b0: banks 5-6; b1: banks 3-4, free once qtb1 is copied out)
                g = b * NCHUNK + c
                tensor.wait_ge(sV[g % VBUFS], (g // VBUFS + 1) * 16)
                tensor.wait_ge(sACT, ACT[f"EXPC{b}_{c}"])
                acc = ps_w if b == 0 else ps_b
                for j in range(CHUNK):
                    t = c * CHUNK + j
                    for nh in range(2):
                        mm = tensor.matmul(
                            out=acc[0:1, nh * 512:(nh + 1) * 512],
                            lhsT=pmat[b][:, t:t + 1],
                            rhs=vt[g % VBUFS][:, j, nh * 512:(nh + 1) * 512],
                            start=(t == 0), stop=(t == NSUB - 1),
                        )
                mm.then_inc(sPE, 1)                      # W{b}C{c}

            def fold_mm(b):
                tensor.wait_ge(sACT, ACT[f"WROW{b}"])
                for dc in range(8):
                    mm = tensor.matmul(
                        out=ps_small[:, 8 + dc:9 + dc],
                        lhsT=w_row[b][0:1, dc * 128:(dc + 1) * 128],
                        rhs=ones_bf[0:1, 0:1],
                        start=True, stop=True,
                    )
                mm.then_inc(sPE, 1)                      # FOLD{b}

            def proj_mm(b):
                tensor.wait_ge(sDVE, DVE[f"WCOL{b}"])
                if b == 0:
                    tensor.wait_ge(sWV, 16)
                    tensor.wait_ge(sACT, ACT["QTSB1"])   # ps_a overwrite guard
                for dc in range(8):
                    for nh in range(2):
                        mm = tensor.matmul(
                            out=ps_a[0:1, nh * 512:(nh + 1) * 512],
                            lhsT=w_col[b][:, dc:dc + 1],
                            rhs=wv_sb[:, dc, nh * 512:(nh + 1) * 512],
                            start=(dc == 0), stop=(dc == 7),
                        )
                mm.then_inc(sPE, 1)                      # PROJ{b}

            # batch-0 tail (fold0/proj0) threads through w1's pacing gaps
            for c in range(NCHUNK):
                w_chunk(0, c)
            z_mm(0)
            w_chunk(1, 0)
            fold_mm(0)
            w_chunk(1, 1)
            proj_mm(0)
            w_chunk(1, 2)
            w_chunk(1, 3)
            z_mm(1)
            fold_mm(1)
            proj_mm(1)

        # ---------- DVE ----------
        @blk.vector
        def _(vector):
            vector.memset(ones_col[:], 1.0).then_inc(sDVE, 1)
            vector.memset(ones_bf[:], 1.0).then_inc(sDVE, 1)

            def qcol(b):
                vector.wait_ge(sPE, PE[f"QF{b}"])
                vector.tensor_copy(out=qcol_sb[b][:], in_=ps_small[:, 16:24]) \
                    .then_inc(sDVE, 1)                                # QCOL{b}

            def small_chain(b, step):
                if step == 0:
                    if b == 0:
                        vector.wait_ge(sW, 64)
                    vector.wait_ge(sPE, PE[f"QPROW{b}"])
                    vector.tensor_add(qpr_sb[b][:], ps_small[0:1, 256:256 + A],
                                      bkq_row[:]).then_inc(sDVE, 1)   # QPRSB{b}
                elif step == 1:
                    vector.wait_ge(sPE, PE[f"QPF{b}"])
                    vector.tensor_copy(out=qp_sb[b][:], in_=ps_small[:, 0:2]) \
                        .then_inc(sDVE, 1)                            # QPSB{b}
                else:
                    vector.wait_ge(sPE, PE[f"QTB{b}"])
                    vector.tensor_copy(out=qtb_sb[b][:], in_=ps_b[:]) \
                        .then_inc(sDVE, 1)                            # QTBSB{b}

            def mult_chunk(b, c):
                g = b * NCHUNK + c
                vector.wait_ge(sK[g % KBUFS], (g // KBUFS + 1) * 16)
                if c == 0:
                    # self-wait: qtb_sb copy completion before reads
                    vector.wait_ge(sDVE, DVE[f"QTBSB{b}"])
                for j in range(CHUNK):
                    t = c * CHUNK + j
                    if j < NACT:
                        if g >= 1:
                            # scratch slot j: previous chunk's reduce done
                            bp, cp = divmod(g - 1, NCHUNK)
                            vector.wait_ge(sACT, ACT[red_ev(bp, cp, j)])
                        vector.tensor_mul(
                            scr[j][:], kt[g % KBUFS][:, j, :], qtb_sb[b][:]
                        ).then_inc(sDVE, 1)               # MUL{b}_{c}_{j}
                    else:
                        # fused dot product on DVE, in-place on the k slice:
                        # out=(k*1)*qt, accum_out=row sum
                        vector.scalar_tensor_tensor(
                            out=kt[g % KBUFS][:, j, :],
                            in0=kt[g % KBUFS][:, j, :], scalar=1.0,
                            in1=qtb_sb[b][:],
                            op0=AL.mult, op1=AL.mult,
                            accum_out=smat[b][:, t:t + 1],
                        ).then_inc(sDVE, 1)               # MUL{b}_{c}_{j}

            def zred(b):
                vector.wait_ge(sACT, ACT[f"EXPC{b}_{NCHUNK - 1}"])
                vector.reduce_sum(zredc[b][:], zpart[b][:],
                                  axis=mybir.AxisListType.X) \
                    .then_inc(sDVE, 1)                                # ZRED{b}

            def tail(b, step):
                if step == 0:
                    vector.wait_ge(sPE, PE[f"Z{b}"])
                    vector.reciprocal(invz[b][:], ps_small[0:1, 4:5]) \
                        .then_inc(sDVE, 1)                            # INVZ{b}
                elif step == 1:
                    vector.wait_ge(sPE, PE[f"FOLD{b}"])
                    vector.tensor_copy(out=w_col[b][:], in_=ps_small[:, 8:16]) \
                        .then_inc(sDVE, 1)                            # WCOL{b}
                else:
                    vector.wait_ge(sPE, PE[f"PROJ{b}"])
                    if b == 0:
                        vector.wait_ge(sBV, 16)
                    vector.tensor_add(o_sb[b][:], ps_a[0:1, :], bv_row[:]) \
                        .then_inc(sDVE, 1)                            # OSB{b}

            qcol(0)
            small_chain(0, 0)
            qcol(1)
            small_chain(0, 1)
            small_chain(0, 2)
            mult_chunk(0, 0)
            small_chain(1, 0)
            mult_chunk(0, 1)
            small_chain(1, 1)
            mult_chunk(0, 2)
            small_chain(1, 2)
            mult_chunk(0, 3)
            mult_chunk(1, 0)
            zred(0)
            mult_chunk(1, 1)
            tail(0, 0)          # INVZ0
            mult_chunk(1, 2)
            tail(0, 1)          # WCOL0
            mult_chunk(1, 3)
            tail(0, 2)          # OSB0
            zred(1)
            tail(1, 0)
            tail(1, 1)
            tail(1, 2)

        # ---------- ACT (scalar) ----------
        @blk.scalar
        def _(scalar):
            def qtsb(b):
                scalar.wait_ge(sPE, PE[f"QT{b}"])
                scalar.mul(qt_sb[b][:], ps_a[0:1, :], 1.0 / 16.0) \
                    .then_inc(sACT, 1)                                # QTSB{b}

            def red_chunk(b, c):
                for j in range(NACT):
                    t = c * CHUNK + j
                    scalar.wait_ge(sDVE, DVE[mult_ev(b, c, j)])
                    scalar.activation(
                        out=scr[j][:], in_=scr[j][:], func=AF.Copy,
                        accum_out=smat[b][:, t:t + 1],
                    ).then_inc(sACT, 1)                   # RED{b}_{c}_{j}

            def expc(b, c):
                # smat chunk writers: ACT reduces (self-order) + DVE stts
                scalar.wait_ge(sACT, ACT[red_ev(b, c, NACT - 1)])
                scalar.wait_ge(sDVE, DVE[mult_ev(b, c, CHUNK - 1)])
                scalar.activation(
                    out=pmat[b][:, c * CHUNK:(c + 1) * CHUNK],
                    in_=smat[b][:, c * CHUNK:(c + 1) * CHUNK], func=AF.Exp,
                    accum_out=zpart[b][:, c:c + 1],
                ).then_inc(sACT, 1)                                   # EXPC{b}_{c}

            def wrow(b):
                scalar.wait_ge(sPE, PE[f"W{b}C{NCHUNK - 1}"])
                scalar.wait_ge(sDVE, DVE[f"INVZ{b}"])
                acc = ps_w if b == 0 else ps_b
                scalar.activation(
                    out=w_row[b][:], in_=acc[0:1, :], func=AF.Copy,
                    bias=0.0, scale=invz[b][0:1, 0:1],
                ).then_inc(sACT, 1)                                   # WROW{b}

            qtsb(0)
            red_chunk(0, 0)
            expc(0, 0)
            qtsb(1)
            for c in range(1, NCHUNK):
                red_chunk(0, c)
                expc(0, c)
            red_chunk(1, 0)
            expc(1, 0)
            red_chunk(1, 1)
            expc(1, 1)
            wrow(0)
            red_chunk(1, 2)
            expc(1, 2)
            red_chunk(1, 3)
            expc(1, 3)
            wrow(1)

    return nc


_NC_CACHE = None


def get_nc():
    global _NC_CACHE
    if _NC_CACHE is None:
        _NC_CACHE = _build_nc()
    return _NC_CACHE


def make_in_maps(q, k, v, W_kq, b_kq, W_v, b_v):
    """Shard full inputs over 8 cores: batch-parallel, weights replicated.
    k, v, W_v are cast to bfloat16 on the host (compute dtype of the
    streaming contractions)."""
    import ml_dtypes

    bf16 = ml_dtypes.bfloat16
    q = np.ascontiguousarray(
        np.asarray(q, dtype=np.float32).reshape(B, E).astype(bf16))
    k = np.ascontiguousarray(np.asarray(k, dtype=np.float32).astype(bf16))
    v = np.ascontiguousarray(np.asarray(v, dtype=np.float32).astype(bf16))
    W_kq32 = np.asarray(W_kq, dtype=np.float32)
    W_kq = np.ascontiguousarray(W_kq32.astype(bf16))
    W_kqT = np.ascontiguousarray(W_kq32.T.astype(bf16))
    b_kq = np.ascontiguousarray(np.asarray(b_kq, dtype=np.float32))
    W_v = np.ascontiguousarray(np.asarray(W_v, dtype=np.float32).astype(bf16))
    b_v = np.ascontiguousarray(np.asarray(b_v, dtype=np.float32))
    in_maps = []
    for i in range(NCORES):
        lo, hi = i * BPC, (i + 1) * BPC
        in_maps.append({
            "q": q[lo:hi],
            "k": k[lo:hi],
            "v": v[lo:hi],
            "W_kq": W_kq,
            "W_kqT": W_kqT,
            "b_kq": b_kq,
            "W_v": W_v,
            "b_v": b_v,
        })
    return in_maps


def kernel(q, k, v, W_kq, b_kq, W_v, b_v):
    from concourse.bass_utils import run_bass_kernel_spmd

    nc = get_nc()
    in_maps = make_in_maps(q, k, v, W_kq, b_kq, W_v, b_v)
    res = run_bass_kernel_spmd(nc, in_maps, core_ids=list(range(NCORES)))
    out = np.concatenate([res.results[i]["out"] for i in range(NCORES)], axis=0)
    return np.ascontiguousarray(out.astype(np.float32))


# revision 56
# speedup vs baseline: 1.0995x; 1.0023x over previous
"""Distributed attention kernel for Trainium2 (8 NeuronCores, SPMD).

Problem: B=16 batches of single-query attention over NK=4096 keys,
EMBED=1024, ATTN=256, with a shared kq projection and a v projection.

Math restructuring (exact up to float reassociation):
  - scores = (q@W_kq + b_kq) @ (k@W_kq + b_kq)^T / 16
           = k @ qt + const            where qt = W_kq @ (W_kq^T q + b_kq) / 16
    (the constant offsets every score equally -> softmax invariant, dropped)
  - out = softmax(scores) @ (v@W_v + b_v)
        = (attn @ v) @ W_v + b_v       (attn sums to 1)
This removes the O(NK*E*E) v-projection and O(NK*E*A) k-projection
entirely; the kernel is HBM-bandwidth bound streaming k and v once.

Sharding: data-parallel over batch, 2 batches per core. k, v, W_v are
cast to bf16 on the host (compute dtype of the streaming contractions,
halves HBM traffic); W_kq is additionally passed pre-transposed.

Token layout is p-major ("(p s) d"): partition p holds NSUB consecutive
token rows, so each chunk DMA is one 16KB-contiguous run per partition
(128 descriptors, not 1024). k and v use the same permutation, and
softmax is globally permutation-invariant, so results are unchanged.

s = k.qt is load-balanced across engines per 128-token tile (per chunk
of 8 tiles: 5 via DVE bf16 multiply (2x mode) + ACT activation-accum
row-sum; 3 via DVE fused scalar_tensor_tensor). w = attn@v and the
projections run on TensorE in bf16 (single pass). Softmax uses
unnormalized exp (scores ~ N(0,1), no overflow in fp32) with 1/Z
folded into the w_row evacuation. Batch 1's DMAs are interleaved with
batch 0's v stream so its k chunks are resident when compute frees up.

Raw bass (not Tile): this toolchain's walrus build rejects >1 embedded
sync-wait per compute instruction, which Tile's scheduler emits; raw
bass uses standalone sequencer waits, with explicit semaphore ticks
precomputed in python (the *_seq tables below).

PSUM bank map (PE-W vs DVE/ACT-R hazards serialized via the sem chain;
bank granularity, not address: concurrent PE-write + DVE/ACT-read of
the SAME bank is a fatal HW error even at different addresses):
  bank 0    : qp_row [0:1,256:512], qp_col [:,0:2], Z [0:1,4:5],
              w fold [:,8:16], q fold [:,16:24]
  banks 1-2 : qt row / out row (disjoint lifetimes)
  banks 3-4 : qt broadcast [128, 1024]
  banks 5-6 : w accumulator row
"""

import contextlib

import numpy as np

try:
    import concourse.bass as bass  # noqa: F401
except ImportError:  # fallback if site path isn't preloaded
    import sys

    sys.path.insert(0, "/opt/trn_rl_repo")

B = 16
NCORES = 8
BPC = B // NCORES  # batches per core
NK = 4096
E = 1024
A = 256
NSUB = NK // 128   # 32 token subtiles of 128
CHUNK = 8          # subtiles per DMA chunk (2 MB in bf16)
NCHUNK = NSUB // CHUNK
KBUFS = 4
VBUFS = 4
NACT = 5           # tiles per chunk reduced on ACT (rest fused on DVE)
# scratch slot j is dedicated to tile position j of each chunk, so a
# chunk's multiplies only wait on the PREVIOUS chunk's reduces


def _build_nc():
    import concourse.bass as bass
    from concourse import mybir

    FP = mybir.dt.float32
    BF = mybir.dt.bfloat16
    AL = mybir.AluOpType
    AF = mybir.ActivationFunctionType

    nc = bass.Bass()
    q_d = nc.declare_dram_parameter("q", [128, BPC * 8], BF, isOutput=False)
    k_d = nc.declare_dram_parameter("k", [BPC, NK, E], BF, isOutput=False)
    v_d = nc.declare_dram_parameter("v", [BPC, NK, E], BF, isOutput=False)
    wkq_d = nc.declare_dram_parameter("W_kq", [E, A], BF, isOutput=False)
    wkqT_d = nc.declare_dram_parameter("W_kqT", [A, E], BF, isOutput=False)
    bkq_d = nc.declare_dram_parameter("b_kq", [A], FP, isOutput=False)
    wv_d = nc.declare_dram_parameter("W_v", [E, E], BF, isOutput=False)
    bv_d = nc.declare_dram_parameter("b_v", [E], FP, isOutput=False)
    out_d = nc.declare_dram_parameter("out", [BPC, E], FP, isOutput=True)

    with contextlib.ExitStack() as st:
        def sb(name, shape, dt=FP):
            return st.enter_context(nc.sbuf_tensor(name, shape, dt))

        # ---- SBUF ----
        wkq_sb = sb("wkq_sb", [128, 8, A], BF)
        wkqT_sb = sb("wkqT_sb", [128, 2, E], BF)
        wv_sb = sb("wv_sb", [128, 8, E], BF)
        q_col = sb("q_col", [128, BPC * 8], BF)
        bkq_row = sb("bkq_row", [1, A])
        bv_row = sb("bv_row", [1, E])
        ones_col = sb("ones_col", [128, 1])        # fp32 (Z rhs)
        ones_bf = sb("ones_bf", [1, 128], BF)      # bf16 (w fold rhs)
        kt = [sb(f"kt{i}", [128, CHUNK, E], BF) for i in range(KBUFS)]
        vt = [sb(f"vt{i}", [128, CHUNK, E], BF) for i in range(VBUFS)]
        scr = [sb(f"scr{i}", [128, E], BF) for i in range(NACT)]
        # per-batch smalls
        qpr_sb = [sb(f"qpr_sb{b}", [1, A], BF) for b in range(BPC)]
        qp_sb = [sb(f"qp_sb{b}", [128, 2], BF) for b in range(BPC)]
        qt_sb = [sb(f"qt_sb{b}", [1, E], BF) for b in range(BPC)]
        qtb_sb = [sb(f"qtb_sb{b}", [128, E], BF) for b in range(BPC)]
        smat = [sb(f"smat{b}", [128, NSUB]) for b in range(BPC)]
        pmat = [sb(f"pmat{b}", [128, NSUB], BF) for b in range(BPC)]
        zpart = [sb(f"zpart{b}", [128, NCHUNK]) for b in range(BPC)]
        zredc = [sb(f"zredc{b}", [128, 1]) for b in range(BPC)]
        invz = [sb(f"invz{b}", [1, 1]) for b in range(BPC)]
        w_row = [sb(f"w_row{b}", [1, E], BF) for b in range(BPC)]
        w_col = [sb(f"w_col{b}", [128, 8], BF) for b in range(BPC)]
        o_sb = [sb(f"o_sb{b}", [1, E]) for b in range(BPC)]

        # ---- PSUM (static bank map) ----
        ps_small = st.enter_context(nc.psum_tensor([128, 512], FP))   # bank 0
        ps_a = st.enter_context(nc.psum_tensor([128, 1024], FP))      # banks 1-2
        ps_b = st.enter_context(nc.psum_tensor([128, 1024], FP))      # banks 3-4
        ps_w = st.enter_context(nc.psum_tensor([128, 1024], FP))      # banks 5-6

        # ---- semaphores ----
        sW = st.enter_context(nc.semaphore("sW"))      # wkq+wkqT+q+bkq -> 64
        sWV = st.enter_context(nc.semaphore("sWV"))    # wv -> 16
        sBV = st.enter_context(nc.semaphore("sBV"))    # bv -> 16
        sK = [st.enter_context(nc.semaphore(f"sK{i}")) for i in range(KBUFS)]
        sV = [st.enter_context(nc.semaphore(f"sV{i}")) for i in range(VBUFS)]
        sOUT = st.enter_context(nc.semaphore("sOUT"))
        sPE = st.enter_context(nc.semaphore("sPE"))
        sDVE = st.enter_context(nc.semaphore("sDVE"))
        sACT = st.enter_context(nc.semaphore("sACT"))

        blk = st.enter_context(nc.Block())

        # ---------- event tick registry ----------
        def ticks(seq):
            return {ev: i + 1 for i, ev in enumerate(seq)}

        pe_seq = ["QPROW0", "QPF0", "QT0", "QTB0",
                  "QPROW1", "QPF1", "QT1", "QTB1"]
        pe_seq += [f"W0C{c}" for c in range(NCHUNK)] + ["Z0"]
        pe_seq += ["W1C0", "FOLD0", "W1C1", "PROJ0", "W1C2", "W1C3",
                   "Z1", "FOLD1", "PROJ1"]
        PE = ticks(pe_seq)

        def mult_ev(b, c, j):
            # DVE inc for tile j of chunk (b, c): mult (j < NACT) or fused stt
            return f"MUL{b}_{c}_{j}"

        def red_ev(b, c, j):
            return f"RED{b}_{c}_{j}"

        dve_seq = ["MS1", "MS2", "QPRSB0", "QPSB0", "QTBSB0"]
        dve_seq += [mult_ev(0, 0, j) for j in range(CHUNK)]
        dve_seq += ["QPRSB1"]
        dve_seq += [mult_ev(0, 1, j) for j in range(CHUNK)]
        dve_seq += ["QPSB1"]
        dve_seq += [mult_ev(0, 2, j) for j in range(CHUNK)]
        dve_seq += ["QTBSB1"]
        dve_seq += [mult_ev(0, 3, j) for j in range(CHUNK)]
        dve_seq += [mult_ev(1, 0, j) for j in range(CHUNK)]
        dve_seq += ["ZRED0"]
        dve_seq += [mult_ev(1, 1, j) for j in range(CHUNK)]
        dve_seq += ["INVZ0"]
        dve_seq += [mult_ev(1, 2, j) for j in range(CHUNK)]
        dve_seq += ["WCOL0"]
        dve_seq += [mult_ev(1, 3, j) for j in range(CHUNK)]
        dve_seq += ["OSB0", "ZRED1", "INVZ1", "WCOL1", "OSB1"]
        DVE = ticks(dve_seq)

        act_seq = ["QTSB0"]
        act_seq += [red_ev(0, 0, j) for j in range(NACT)] + ["EXPC0_0", "QTSB1"]
        for c in range(1, NCHUNK):
            act_seq += [red_ev(0, c, j) for j in range(NACT)] + [f"EXPC0_{c}"]
        act_seq += [red_ev(1, 0, j) for j in range(NACT)] + ["EXPC1_0"]
        act_seq += [red_ev(1, 1, j) for j in range(NACT)] + ["EXPC1_1", "WROW0"]
        act_seq += [red_ev(1, 2, j) for j in range(NACT)] + ["EXPC1_2"]
        act_seq += [red_ev(1, 3, j) for j in range(NACT)] + ["EXPC1_3", "WROW1"]
        ACT = ticks(act_seq)

        # ---------- SYNC: all DMAs ----------
        @blk.sync
        def _(sync):
            sync.dma_start(
                out=wkq_sb[:], in_=wkq_d[:].rearrange("(dc p) a -> p dc a", p=128)
            ).then_inc(sW, 16)
            sync.dma_start(
                out=wkqT_sb[:], in_=wkqT_d[:].rearrange("(ac p) d -> p ac d", p=128)
            ).then_inc(sW, 16)
            sync.dma_start(out=q_col[:], in_=q_d[:]).then_inc(sW, 16)
            sync.dma_start(out=bkq_row[:], in_=bkq_d[:][None, :]).then_inc(sW, 16)
            sync.dma_start(out=bv_row[:], in_=bv_d[:][None, :]).then_inc(sBV, 16)

            def kdma(b, c):
                g = b * NCHUNK + c
                if g >= KBUFS:
                    gp = g - KBUFS
                    bp, cp = divmod(gp, NCHUNK)
                    sync.wait_ge(sDVE, DVE[mult_ev(bp, cp, CHUNK - 1)])
                k_b = k_d[:][b].rearrange("(p s) d -> p s d", p=128)
                sync.dma_start(
                    out=kt[g % KBUFS][:],
                    in_=k_b[:, c * CHUNK:(c + 1) * CHUNK, :],
                ).then_inc(sK[g % KBUFS], 16)

            def vdma(b, c):
                g = b * NCHUNK + c
                if g >= VBUFS:
                    gp = g - VBUFS
                    bp, cp = divmod(gp, NCHUNK)
                    sync.wait_ge(sPE, PE[f"W{bp}C{cp}"])
                v_b = v_d[:][b].rearrange("(p s) d -> p s d", p=128)
                sync.dma_start(
                    out=vt[g % VBUFS][:],
                    in_=v_b[:, c * CHUNK:(c + 1) * CHUNK, :],
                ).then_inc(sV[g % VBUFS], 16)

            # online softmax: k and v chunks interleave per batch (w-MMs
            # consume each v chunk right after its k chunk's exp)
            kdma(0, 0)
            kdma(0, 1)
            vdma(0, 0)
            kdma(0, 2)
            vdma(0, 1)
            kdma(0, 3)
            vdma(0, 2)
            vdma(0, 3)
            sync.dma_start(
                out=wv_sb[:], in_=wv_d[:].rearrange("(dc p) e -> p dc e", p=128)
            ).then_inc(sWV, 16)
            kdma(1, 0)
            kdma(1, 1)
            vdma(1, 0)
            kdma(1, 2)
            vdma(1, 1)
            kdma(1, 3)
            vdma(1, 2)
            vdma(1, 3)

            for b in range(BPC):
                sync.wait_ge(sDVE, DVE[f"OSB{b}"])
                sync.dma_start(out=out_d[:][b:b + 1, :], in_=o_sb[b][:]).then_inc(
                    sOUT, 16)
            sync.wait_ge(sOUT, BPC * 16)

        # ---------- PE ----------
        @blk.tensor
        def _(tensor):
            tensor.wait_ge(sW, 64)
            tensor.wait_ge(sDVE, DVE["MS2"])  # ones tiles ready
            def qprow(b):
                if b > 0:
                    # bank-0 safety: prior batch's bank-0 reads done
                    tensor.wait_ge(sDVE, DVE[f"QPSB{b - 1}"])
                for dc in range(8):
                    mm = tensor.matmul(
                        out=ps_small[0:1, 256:256 + A],
                        lhsT=q_col[:, b * 8 + dc:b * 8 + dc + 1],
                        rhs=wkq_sb[:, dc, :],
                        start=(dc == 0), stop=(dc == 7),
                    )
                mm.then_inc(sPE, 1)                      # QPROW{b}

            def qpf(b):
                tensor.wait_ge(sDVE, DVE[f"QPRSB{b}"])
                for c2 in range(2):
                    mm = tensor.matmul(
                        out=ps_small[:, c2:c2 + 1],
                        lhsT=qpr_sb[b][0:1, c2 * 128:(c2 + 1) * 128],
                        rhs=ones_bf[0:1, 0:1],
                        start=True, stop=True,
                    )
                mm.then_inc(sPE, 1)                      # QPF{b}

            def qt_mm(b):
                tensor.wait_ge(sDVE, DVE[f"QPSB{b}"])
                if b > 0:
                    tensor.wait_ge(sACT, ACT[f"QTSB{b - 1}"])
                for ac in range(2):
                    for nh in range(2):
                        mm = tensor.matmul(
                            out=ps_a[0:1, nh * 512:(nh + 1) * 512],
                            lhsT=qp_sb[b][:, ac:ac + 1],
                            rhs=wkqT_sb[:, ac, nh * 512:(nh + 1) * 512],
                            start=(ac == 0), stop=(ac == 1),
                        )
                mm.then_inc(sPE, 1)                      # QT{b}

            def qtb_mm(b):
                tensor.wait_ge(sACT, ACT[f"QTSB{b}"])
                for nh in range(2):
                    mm = tensor.matmul(
                        out=ps_b[:, nh * 512:(nh + 1) * 512],
                        lhsT=ones_bf[:],
                        rhs=qt_sb[b][0:1, nh * 512:(nh + 1) * 512],
                        start=True, stop=True,
                    )
                mm.then_inc(sPE, 1)                      # QTB{b}

            qprow(0)
            qpf(0)
            qt_mm(0)
            qtb_mm(0)
            qprow(1)
            qpf(1)
            qt_mm(1)
            qtb_mm(1)

            def z_mm(b):
                tensor.wait_ge(sDVE, DVE[f"ZRED{b}"])
                tensor.matmul(
                    out=ps_small[0:1, 4:5], lhsT=zredc[b][:], rhs=ones_col[:],
                    start=True, stop=True,
                ).then_inc(sPE, 1)                       # Z{b}

            def w_chunk(b, c):
                # online accumulation: batch b's w sums into its own psum
                # (b0: banks 5-6; b1: banks 3-4, free once qtb1 is copied out)
                g = b * NCHUNK + c
                tensor.wait_ge(sV[g % VBUFS], (g // VBUFS + 1) * 16)
                tensor.wait_ge(sACT, ACT[f"EXPC{b}_{c}"])
                acc = ps_w if b == 0 else ps_b
                for j in range(CHUNK):
                    t = c * CHUNK + j
                    for nh in range(2):
                        mm = tensor.matmul(
                            out=acc[0:1, nh * 512:(nh + 1) * 512],
                            lhsT=pmat[b][:, t:t + 1],
                            rhs=vt[g % VBUFS][:, j, nh * 512:(nh + 1) * 512],
                            start=(t == 0), stop=(t == NSUB - 1),
                        )
                mm.then_inc(sPE, 1)                      # W{b}C{c}

            def fold_mm(b):
                tensor.wait_ge(sACT, ACT[f"WROW{b}"])
                for dc in range(8):
                    mm = tensor.matmul(
                        out=ps_small[:, 8 + dc:9 + dc],
                        lhsT=w_row[b][0:1, dc * 128:(dc + 1) * 128],
                        rhs=ones_bf[0:1, 0:1],
                        start=True, stop=True,
                    )
                mm.then_inc(sPE, 1)                      # FOLD{b}

            def proj_mm(b):
                tensor.wait_ge(sDVE, DVE[f"WCOL{b}"])
                if b == 0:
                    tensor.wait_ge(sWV, 16)
                    tensor.wait_ge(sACT, ACT["QTSB1"])   # ps_a overwrite guard
                for dc in range(8):
                    for nh in range(2):
                        mm = tensor.matmul(
                            out=ps_a[0:1, nh * 512:(nh + 1) * 512],
                            lhsT=w_col[b][:, dc:dc + 1],
                            rhs=wv_sb[:, dc, nh * 512:(nh + 1) * 512],
                            start=(dc == 0), stop=(dc == 7),
                        )
                mm.then_inc(sPE, 1)                      # PROJ{b}

            # batch-0 tail (fold0/proj0) threads through w1's pacing gaps
            for c in range(NCHUNK):
                w_chunk(0, c)
            z_mm(0)
            w_chunk(1, 0)
            fold_mm(0)
            w_chunk(1, 1)
            proj_mm(0)
            w_chunk(1, 2)
            w_chunk(1, 3)
            z_mm(1)
            fold_mm(1)
            proj_mm(1)

        # ---------- DVE ----------
        @blk.vector
        def _(vector):
            vector.memset(ones_col[:], 1.0).then_inc(sDVE, 1)
            vector.memset(ones_bf[:], 1.0).then_inc(sDVE, 1)

            def small_chain(b, step):
                if step == 0:
                    if b == 0:
                        vector.wait_ge(sW, 64)
                    vector.wait_ge(sPE, PE[f"QPROW{b}"])
                    vector.tensor_add(qpr_sb[b][:], ps_small[0:1, 256:256 + A],
                                      bkq_row[:]).then_inc(sDVE, 1)   # QPRSB{b}
                elif step == 1:
                    vector.wait_ge(sPE, PE[f"QPF{b}"])
                    vector.tensor_copy(out=qp_sb[b][:], in_=ps_small[:, 0:2]) \
                        .then_inc(sDVE, 1)                            # QPSB{b}
                else:
                    vector.wait_ge(sPE, PE[f"QTB{b}"])
                    vector.tensor_copy(out=qtb_sb[b][:], in_=ps_b[:]) \
                        .then_inc(sDVE, 1)                            # QTBSB{b}

            def mult_chunk(b, c):
                g = b * NCHUNK + c
                vector.wait_ge(sK[g % KBUFS], (g // KBUFS + 1) * 16)
                if c == 0:
                    # self-wait: qtb_sb copy completion before reads
                    vector.wait_ge(sDVE, DVE[f"QTBSB{b}"])
                for j in range(CHUNK):
                    t = c * CHUNK + j
                    if j < NACT:
                        if g >= 1:
                            # scratch slot j: previous chunk's reduce done
                            bp, cp = divmod(g - 1, NCHUNK)
                            vector.wait_ge(sACT, ACT[red_ev(bp, cp, j)])
                        vector.tensor_mul(
                            scr[j][:], kt[g % KBUFS][:, j, :], qtb_sb[b][:]
                        ).then_inc(sDVE, 1)               # MUL{b}_{c}_{j}
                    else:
                        # fused dot product on DVE, in-place on the k slice:
                        # out=(k*1)*qt, accum_out=row sum
                        vector.scalar_tensor_tensor(
                            out=kt[g % KBUFS][:, j, :],
                            in0=kt[g % KBUFS][:, j, :], scalar=1.0,
                            in1=qtb_sb[b][:],
                            op0=AL.mult, op1=AL.mult,
                            accum_out=smat[b][:, t:t + 1],
                        ).then_inc(sDVE, 1)               # MUL{b}_{c}_{j}

            def zred(b):
                vector.wait_ge(sACT, ACT[f"EXPC{b}_{NCHUNK - 1}"])
                vector.reduce_sum(zredc[b][:], zpart[b][:],
                                  axis=mybir.AxisListType.X) \
                    .then_inc(sDVE, 1)                                # ZRED{b}

            def tail(b, step):
                if step == 0:
                    vector.wait_ge(sPE, PE[f"Z{b}"])
                    vector.reciprocal(invz[b][:], ps_small[0:1, 4:5]) \
                        .then_inc(sDVE, 1)                            # INVZ{b}
                elif step == 1:
                    vector.wait_ge(sPE, PE[f"FOLD{b}"])
                    vector.tensor_copy(out=w_col[b][:], in_=ps_small[:, 8:16]) \
                        .then_inc(sDVE, 1)                            # WCOL{b}
                else:
                    vector.wait_ge(sPE, PE[f"PROJ{b}"])
                    if b == 0:
                        vector.wait_ge(sBV, 16)
                    vector.tensor_add(o_sb[b][:], ps_a[0:1, :], bv_row[:]) \
                        .then_inc(sDVE, 1)                            # OSB{b}

            small_chain(0, 0)
            small_chain(0, 1)
            small_chain(0, 2)
            mult_chunk(0, 0)
            small_chain(1, 0)
            mult_chunk(0, 1)
            small_chain(1, 1)
            mult_chunk(0, 2)
            small_chain(1, 2)
            mult_chunk(0, 3)
            mult_chunk(1, 0)
            zred(0)
            mult_chunk(1, 1)
            tail(0, 0)          # INVZ0
            mult_chunk(1, 2)
            tail(0, 1)          # WCOL0
            mult_chunk(1, 3)
            tail(0, 2)          # OSB0
            zred(1)
            tail(1, 0)
            tail(1, 1)
            tail(1, 2)

        # ---------- ACT (scalar) ----------
        @blk.scalar
        def _(scalar):
            def qtsb(b):
                scalar.wait_ge(sPE, PE[f"QT{b}"])
                scalar.mul(qt_sb[b][:], ps_a[0:1, :], 1.0 / 16.0) \
                    .then_inc(sACT, 1)                                # QTSB{b}

            def red_chunk(b, c):
                for j in range(NACT):
                    t = c * CHUNK + j
                    scalar.wait_ge(sDVE, DVE[mult_ev(b, c, j)])
                    scalar.activation(
                        out=scr[j][:], in_=scr[j][:], func=AF.Copy,
                        accum_out=smat[b][:, t:t + 1],
                    ).then_inc(sACT, 1)                   # RED{b}_{c}_{j}

            def expc(b, c):
                # smat chunk writers: ACT reduces (self-order) + DVE stts
                scalar.wait_ge(sACT, ACT[red_ev(b, c, NACT - 1)])
                scalar.wait_ge(sDVE, DVE[mult_ev(b, c, CHUNK - 1)])
                scalar.activation(
                    out=pmat[b][:, c * CHUNK:(c + 1) * CHUNK],
                    in_=smat[b][:, c * CHUNK:(c + 1) * CHUNK], func=AF.Exp,
                    accum_out=zpart[b][:, c:c + 1],
                ).then_inc(sACT, 1)                                   # EXPC{b}_{c}

            def wrow(b):
                scalar.wait_ge(sPE, PE[f"W{b}C{NCHUNK - 1}"])
                scalar.wait_ge(sDVE, DVE[f"INVZ{b}"])
                acc = ps_w if b == 0 else ps_b
                scalar.activation(
                    out=w_row[b][:], in_=acc[0:1, :], func=AF.Copy,
                    bias=0.0, scale=invz[b][0:1, 0:1],
                ).then_inc(sACT, 1)                                   # WROW{b}

            qtsb(0)
            red_chunk(0, 0)
            expc(0, 0)
            qtsb(1)
            for c in range(1, NCHUNK):
                red_chunk(0, c)
                expc(0, c)
            red_chunk(1, 0)
            expc(1, 0)
            red_chunk(1, 1)
            expc(1, 1)
            wrow(0)
            red_chunk(1, 2)
            expc(1, 2)
            red_chunk(1, 3)
            expc(1, 3)
            wrow(1)

    return nc


_NC_CACHE = None


def get_nc():
    global _NC_CACHE
    if _NC_CACHE is None:
        _NC_CACHE = _build_nc()
    return _NC_CACHE


def make_in_maps(q, k, v, W_kq, b_kq, W_v, b_v):
    """Shard full inputs over 8 cores: batch-parallel, weights replicated.
    k, v, W_v are cast to bfloat16 on the host (compute dtype of the
    streaming contractions)."""
    import ml_dtypes

    bf16 = ml_dtypes.bfloat16
    # q pre-folded to column layout: qc[p, b, c] = q[b, c*128+p]
    q = np.asarray(q, dtype=np.float32).reshape(B, E)
    qc = np.ascontiguousarray(
        q.reshape(B, 8, 128).transpose(2, 0, 1).astype(bf16))  # [128, B, 8]
    k = np.ascontiguousarray(np.asarray(k, dtype=np.float32).astype(bf16))
    v = np.ascontiguousarray(np.asarray(v, dtype=np.float32).astype(bf16))
    W_kq32 = np.asarray(W_kq, dtype=np.float32)
    W_kq = np.ascontiguousarray(W_kq32.astype(bf16))
    W_kqT = np.ascontiguousarray(W_kq32.T.astype(bf16))
    b_kq = np.ascontiguousarray(np.asarray(b_kq, dtype=np.float32))
    W_v = np.ascontiguousarray(np.asarray(W_v, dtype=np.float32).astype(bf16))
    b_v = np.ascontiguousarray(np.asarray(b_v, dtype=np.float32))
    in_maps = []
    for i in range(NCORES):
        lo, hi = i * BPC, (i + 1) * BPC
        in_maps.append({
            "q": np.ascontiguousarray(qc[:, lo:hi, :]).reshape(128, BPC * 8),
            "k": k[lo:hi],
            "v": v[lo:hi],
            "W_kq": W_kq,
            "W_kqT": W_kqT,
            "b_kq": b_kq,
            "W_v": W_v,
            "b_v": b_v,
        })
    return in_maps


def kernel(q, k, v, W_kq, b_kq, W_v, b_v):
    from concourse.bass_utils import run_bass_kernel_spmd

    nc = get_nc()
    in_maps = make_in_maps(q, k, v, W_kq, b_kq, W_v, b_v)
    res = run_bass_kernel_spmd(nc, in_maps, core_ids=list(range(NCORES)))
    out = np.concatenate([res.results[i]["out"] for i in range(NCORES)], axis=0)
    return np.ascontiguousarray(out.astype(np.float32))


# revision 58
# speedup vs baseline: 1.2671x; 1.1524x over previous
"""Distributed attention kernel for Trainium2 (8 NeuronCores, SPMD).

Problem: B=16 batches of single-query attention over NK=4096 keys,
EMBED=1024, ATTN=256, with a shared kq projection and a v projection.

Math restructuring (exact up to float reassociation):
  - scores = (q@W_kq + b_kq) @ (k@W_kq + b_kq)^T / 16
           = k @ qt + const            where qt = W_kq @ (W_kq^T q + b_kq) / 16
    (the constant offsets every score equally -> softmax invariant, dropped)
  - out = softmax(scores) @ (v@W_v + b_v)
        = (attn @ v) @ W_v + b_v       (attn sums to 1)
This removes the O(NK*E*E) v-projection and O(NK*E*A) k-projection
entirely; the kernel is HBM-bandwidth bound streaming k and v once.

Sharding: data-parallel over batch, 2 batches per core. k, v, W_v are
cast to bf16 on the host (compute dtype of the streaming contractions,
halves HBM traffic); W_kq is additionally passed pre-transposed.

Token layout is p-major ("(p s) d"): partition p holds NSUB consecutive
token rows, so each chunk DMA is one 16KB-contiguous run per partition
(128 descriptors, not 1024). k and v use the same permutation, and
softmax is globally permutation-invariant, so results are unchanged.

s = k.qt is load-balanced across engines per 128-token tile (per chunk
of 8 tiles: 5 via DVE bf16 multiply (2x mode) + ACT activation-accum
row-sum; 3 via DVE fused scalar_tensor_tensor). w = attn@v and the
projections run on TensorE in bf16 (single pass). Softmax uses
unnormalized exp (scores ~ N(0,1), no overflow in fp32) with 1/Z
folded into the w_row evacuation. Batch 1's DMAs are interleaved with
batch 0's v stream so its k chunks are resident when compute frees up.

Raw bass (not Tile): this toolchain's walrus build rejects >1 embedded
sync-wait per compute instruction, which Tile's scheduler emits; raw
bass uses standalone sequencer waits, with explicit semaphore ticks
precomputed in python (the *_seq tables below).

PSUM bank map (PE-W vs DVE/ACT-R hazards serialized via the sem chain;
bank granularity, not address: concurrent PE-write + DVE/ACT-read of
the SAME bank is a fatal HW error even at different addresses):
  bank 0    : qp_row [0:1,256:512], qp_col [:,0:2], Z [0:1,4:5],
              w fold [:,8:16], q fold [:,16:24]
  banks 1-2 : qt row / out row (disjoint lifetimes)
  banks 3-4 : qt broadcast [128, 1024]
  banks 5-6 : w accumulator row
"""

import contextlib

import numpy as np

try:
    import concourse.bass as bass  # noqa: F401
except ImportError:  # fallback if site path isn't preloaded
    import sys

    sys.path.insert(0, "/opt/trn_rl_repo")

B = 16
NCORES = 8
BPC = B // NCORES  # batches per core
NK = 4096
E = 1024
A = 256
NSUB = NK // 128   # 32 token subtiles of 128
CHUNK = 8          # subtiles per DMA chunk (2 MB in bf16)
NCHUNK = NSUB // CHUNK
KBUFS = 4
VBUFS = 4
NACT = 5           # tiles per chunk reduced on ACT (rest fused on DVE)
# scratch slot j is dedicated to tile position j of each chunk, so a
# chunk's multiplies only wait on the PREVIOUS chunk's reduces


def _build_nc():
    import concourse.bass as bass
    from concourse import mybir

    FP = mybir.dt.float32
    BF = mybir.dt.bfloat16
    AL = mybir.AluOpType
    AF = mybir.ActivationFunctionType

    nc = bass.Bass()
    q_d = nc.declare_dram_parameter("q", [128, BPC * 8], BF, isOutput=False)
    k_d = nc.declare_dram_parameter("k", [BPC, NK, E], BF, isOutput=False)
    v_d = nc.declare_dram_parameter("v", [BPC, NK, E], BF, isOutput=False)
    wkq_d = nc.declare_dram_parameter("W_kq", [E, A], BF, isOutput=False)
    wkqT_d = nc.declare_dram_parameter("W_kqT", [A, E], BF, isOutput=False)
    bkq_d = nc.declare_dram_parameter("b_kq", [A], FP, isOutput=False)
    wv_d = nc.declare_dram_parameter("W_v", [E, E], BF, isOutput=False)
    bv_d = nc.declare_dram_parameter("b_v", [E], FP, isOutput=False)
    out_d = nc.declare_dram_parameter("out", [BPC, E], FP, isOutput=True)

    with contextlib.ExitStack() as st:
        def sb(name, shape, dt=FP):
            return st.enter_context(nc.sbuf_tensor(name, shape, dt))

        # ---- SBUF ----
        wkq_sb = sb("wkq_sb", [128, 8, A], BF)
        wkqT_sb = sb("wkqT_sb", [128, 2, E], BF)
        wv_sb = sb("wv_sb", [128, 8, E], BF)
        q_col = sb("q_col", [128, BPC * 8], BF)
        bkq_row = sb("bkq_row", [1, A])
        bv_row = sb("bv_row", [1, E])
        ones_col = sb("ones_col", [128, 1])        # fp32 (Z rhs)
        ones_bf = sb("ones_bf", [1, 128], BF)      # bf16 (w fold rhs)
        kt = [sb(f"kt{i}", [128, CHUNK, E], BF) for i in range(KBUFS)]
        vt = [sb(f"vt{i}", [128, CHUNK, E], BF) for i in range(VBUFS)]
        scr = [sb(f"scr{i}", [128, E], BF) for i in range(NACT)]
        # per-batch smalls
        qpr_sb = [sb(f"qpr_sb{b}", [1, A], BF) for b in range(BPC)]
        qp_sb = [sb(f"qp_sb{b}", [128, 2], BF) for b in range(BPC)]
        qt_sb = [sb(f"qt_sb{b}", [1, E], BF) for b in range(BPC)]
        qtb_sb = [sb(f"qtb_sb{b}", [128, E], BF) for b in range(BPC)]
        smat = [sb(f"smat{b}", [128, NSUB]) for b in range(BPC)]
        pmat = [sb(f"pmat{b}", [128, NSUB], BF) for b in range(BPC)]
        zpart = [sb(f"zpart{b}", [128, NCHUNK]) for b in range(BPC)]
        zredc = [sb(f"zredc{b}", [128, 1]) for b in range(BPC)]
        invz = [sb(f"invz{b}", [1, 1]) for b in range(BPC)]
        w_row = [sb(f"w_row{b}", [1, E], BF) for b in range(BPC)]
        w_col = [sb(f"w_col{b}", [128, 8], BF) for b in range(BPC)]
        o_sb = [sb(f"o_sb{b}", [1, E]) for b in range(BPC)]

        # ---- PSUM (static bank map) ----
        ps_small = st.enter_context(nc.psum_tensor([128, 512], FP))   # bank 0
        ps_a = st.enter_context(nc.psum_tensor([128, 1024], FP))      # banks 1-2
        ps_b = st.enter_context(nc.psum_tensor([128, 1024], FP))      # banks 3-4
        ps_w = st.enter_context(nc.psum_tensor([128, 1024], FP))      # banks 5-6

        # ---- semaphores ----
        sW = st.enter_context(nc.semaphore("sW"))      # wkq+wkqT+q+bkq -> 64
        sWV = st.enter_context(nc.semaphore("sWV"))    # wv -> 16
        sBV = st.enter_context(nc.semaphore("sBV"))    # bv -> 16
        sK = [st.enter_context(nc.semaphore(f"sK{i}")) for i in range(KBUFS)]
        sV = [st.enter_context(nc.semaphore(f"sV{i}")) for i in range(VBUFS)]
        sOUT = st.enter_context(nc.semaphore("sOUT"))
        sPE = st.enter_context(nc.semaphore("sPE"))
        sDVE = st.enter_context(nc.semaphore("sDVE"))
        sACT = st.enter_context(nc.semaphore("sACT"))

        blk = st.enter_context(nc.Block())

        # ---------- event tick registry ----------
        def ticks(seq):
            return {ev: i + 1 for i, ev in enumerate(seq)}

        pe_seq = ["QPROW0", "QPF0", "QT0", "QTB0",
                  "QPROW1", "QPF1", "QT1"]
        pe_seq += ["W0C0", "W0C1", "QTB1", "W0C2", "W0C3", "Z0"]
        pe_seq += ["W1C0", "FOLD0", "W1C1", "PROJ0", "W1C2", "W1C3",
                   "Z1", "FOLD1", "PROJ1"]
        PE = ticks(pe_seq)

        def mult_ev(b, c, j):
            # DVE inc for tile j of chunk (b, c): mult (j < NACT) or fused stt
            return f"MUL{b}_{c}_{j}"

        def red_ev(b, c, j):
            return f"RED{b}_{c}_{j}"

        dve_seq = ["MS1", "MS2", "QPRSB0", "QPSB0", "QTBSB0"]
        dve_seq += [mult_ev(0, 0, j) for j in range(CHUNK)]
        dve_seq += ["QPRSB1"]
        dve_seq += [mult_ev(0, 1, j) for j in range(CHUNK)]
        dve_seq += ["QPSB1"]
        dve_seq += [mult_ev(0, 2, j) for j in range(CHUNK)]
        dve_seq += [mult_ev(0, 3, j) for j in range(CHUNK)]
        dve_seq += ["QTBSB1"]
        dve_seq += [mult_ev(1, 0, j) for j in range(CHUNK)]
        dve_seq += ["ZRED0"]
        dve_seq += [mult_ev(1, 1, j) for j in range(CHUNK)]
        dve_seq += ["INVZ0"]
        dve_seq += [mult_ev(1, 2, j) for j in range(CHUNK)]
        dve_seq += ["WCOL0"]
        dve_seq += [mult_ev(1, 3, j) for j in range(CHUNK)]
        dve_seq += ["OSB0", "ZRED1", "INVZ1", "WCOL1", "OSB1"]
        DVE = ticks(dve_seq)

        act_seq = ["QTSB0"]
        act_seq += [red_ev(0, 0, j) for j in range(NACT)] + ["EXPC0_0"]
        act_seq += [red_ev(0, 1, j) for j in range(NACT)] + ["EXPC0_1"]
        act_seq += [red_ev(0, 2, j) for j in range(NACT)] + ["EXPC0_2", "QTSB1"]
        act_seq += [red_ev(0, 3, j) for j in range(NACT)] + ["EXPC0_3"]
        act_seq += [red_ev(1, 0, j) for j in range(NACT)] + ["EXPC1_0"]
        act_seq += [red_ev(1, 1, j) for j in range(NACT)] + ["EXPC1_1", "WROW0"]
        act_seq += [red_ev(1, 2, j) for j in range(NACT)] + ["EXPC1_2"]
        act_seq += [red_ev(1, 3, j) for j in range(NACT)] + ["EXPC1_3", "WROW1"]
        ACT = ticks(act_seq)

        # ---------- SYNC: all DMAs ----------
        @blk.sync
        def _(sync):
            sync.dma_start(
                out=wkq_sb[:], in_=wkq_d[:].rearrange("(dc p) a -> p dc a", p=128)
            ).then_inc(sW, 16)
            sync.dma_start(
                out=wkqT_sb[:], in_=wkqT_d[:].rearrange("(ac p) d -> p ac d", p=128)
            ).then_inc(sW, 16)
            sync.dma_start(out=q_col[:], in_=q_d[:]).then_inc(sW, 16)
            sync.dma_start(out=bkq_row[:], in_=bkq_d[:][None, :]).then_inc(sW, 16)
            sync.dma_start(out=bv_row[:], in_=bv_d[:][None, :]).then_inc(sBV, 16)

            def kdma(b, c):
                g = b * NCHUNK + c
                if g >= KBUFS:
                    gp = g - KBUFS
                    bp, cp = divmod(gp, NCHUNK)
                    sync.wait_ge(sDVE, DVE[mult_ev(bp, cp, CHUNK - 1)])
                k_b = k_d[:][b].rearrange("(p s) d -> p s d", p=128)
                sync.dma_start(
                    out=kt[g % KBUFS][:],
                    in_=k_b[:, c * CHUNK:(c + 1) * CHUNK, :],
                ).then_inc(sK[g % KBUFS], 16)

            def vdma(b, c):
                g = b * NCHUNK + c
                if g >= VBUFS:
                    gp = g - VBUFS
                    bp, cp = divmod(gp, NCHUNK)
                    sync.wait_ge(sPE, PE[f"W{bp}C{cp}"])
                v_b = v_d[:][b].rearrange("(p s) d -> p s d", p=128)
                sync.dma_start(
                    out=vt[g % VBUFS][:],
                    in_=v_b[:, c * CHUNK:(c + 1) * CHUNK, :],
                ).then_inc(sV[g % VBUFS], 16)

            # online softmax: k and v chunks interleave per batch (w-MMs
            # consume each v chunk right after its k chunk's exp)
            kdma(0, 0)
            kdma(0, 1)
            vdma(0, 0)
            kdma(0, 2)
            vdma(0, 1)
            kdma(0, 3)
            vdma(0, 2)
            vdma(0, 3)
            sync.dma_start(
                out=wv_sb[:], in_=wv_d[:].rearrange("(dc p) e -> p dc e", p=128)
            ).then_inc(sWV, 16)
            kdma(1, 0)
            kdma(1, 1)
            vdma(1, 0)
            kdma(1, 2)
            vdma(1, 1)
            kdma(1, 3)
            vdma(1, 2)
            vdma(1, 3)

            for b in range(BPC):
                sync.wait_ge(sDVE, DVE[f"OSB{b}"])
                sync.dma_start(out=out_d[:][b:b + 1, :], in_=o_sb[b][:]).then_inc(
                    sOUT, 16)
            sync.wait_ge(sOUT, BPC * 16)

        # ---------- PE ----------
        @blk.tensor
        def _(tensor):
            tensor.wait_ge(sW, 64)
            tensor.wait_ge(sDVE, DVE["MS2"])  # ones tiles ready
            def qprow(b):
                if b > 0:
                    # bank-0 safety: prior batch's bank-0 reads done
                    tensor.wait_ge(sDVE, DVE[f"QPSB{b - 1}"])
                for dc in range(8):
                    mm = tensor.matmul(
                        out=ps_small[0:1, 256:256 + A],
                        lhsT=q_col[:, b * 8 + dc:b * 8 + dc + 1],
                        rhs=wkq_sb[:, dc, :],
                        start=(dc == 0), stop=(dc == 7),
                    )
                mm.then_inc(sPE, 1)                      # QPROW{b}

            def qpf(b):
                tensor.wait_ge(sDVE, DVE[f"QPRSB{b}"])
                for c2 in range(2):
                    mm = tensor.matmul(
                        out=ps_small[:, c2:c2 + 1],
                        lhsT=qpr_sb[b][0:1, c2 * 128:(c2 + 1) * 128],
                        rhs=ones_bf[0:1, 0:1],
                        start=True, stop=True,
                    )
                mm.then_inc(sPE, 1)                      # QPF{b}

            def qt_mm(b):
                tensor.wait_ge(sDVE, DVE[f"QPSB{b}"])
                if b > 0:
                    tensor.wait_ge(sACT, ACT[f"QTSB{b - 1}"])
                for ac in range(2):
                    for nh in range(2):
                        mm = tensor.matmul(
                            out=ps_a[0:1, nh * 512:(nh + 1) * 512],
                            lhsT=qp_sb[b][:, ac:ac + 1],
                            rhs=wkqT_sb[:, ac, nh * 512:(nh + 1) * 512],
                            start=(ac == 0), stop=(ac == 1),
                        )
                mm.then_inc(sPE, 1)                      # QT{b}

            def qtb_mm(b):
                tensor.wait_ge(sACT, ACT[f"QTSB{b}"])
                for nh in range(2):
                    mm = tensor.matmul(
                        out=ps_b[:, nh * 512:(nh + 1) * 512],
                        lhsT=ones_bf[:],
                        rhs=qt_sb[b][0:1, nh * 512:(nh + 1) * 512],
                        start=True, stop=True,
                    )
                mm.then_inc(sPE, 1)                      # QTB{b}

            qprow(0)
            qpf(0)
            qt_mm(0)
            qtb_mm(0)
            qprow(1)
            qpf(1)
            qt_mm(1)

            def z_mm(b):
                tensor.wait_ge(sDVE, DVE[f"ZRED{b}"])
                tensor.matmul(
                    out=ps_small[0:1, 4:5], lhsT=zredc[b][:], rhs=ones_col[:],
                    start=True, stop=True,
                ).then_inc(sPE, 1)                       # Z{b}

            def w_chunk(b, c):
                # online accumulation: batch b's w sums into its own psum
                # (b0: banks 5-6; b1: banks 3-4, free once qtb1 is copied out)
                g = b * NCHUNK + c
                tensor.wait_ge(sV[g % VBUFS], (g // VBUFS + 1) * 16)
                tensor.wait_ge(sACT, ACT[f"EXPC{b}_{c}"])
                acc = ps_w if b == 0 else ps_b
                for j in range(CHUNK):
                    t = c * CHUNK + j
                    for nh in range(2):
                        mm = tensor.matmul(
                            out=acc[0:1, nh * 512:(nh + 1) * 512],
                            lhsT=pmat[b][:, t:t + 1],
                            rhs=vt[g % VBUFS][:, j, nh * 512:(nh + 1) * 512],
                            start=(t == 0), stop=(t == NSUB - 1),
                        )
                mm.then_inc(sPE, 1)                      # W{b}C{c}

            def fold_mm(b):
                tensor.wait_ge(sACT, ACT[f"WROW{b}"])
                for dc in range(8):
                    mm = tensor.matmul(
                        out=ps_small[:, 8 + dc:9 + dc],
                        lhsT=w_row[b][0:1, dc * 128:(dc + 1) * 128],
                        rhs=ones_bf[0:1, 0:1],
                        start=True, stop=True,
                    )
                mm.then_inc(sPE, 1)                      # FOLD{b}

            def proj_mm(b):
                tensor.wait_ge(sDVE, DVE[f"WCOL{b}"])
                if b == 0:
                    tensor.wait_ge(sWV, 16)
                    tensor.wait_ge(sACT, ACT["QTSB1"])   # ps_a overwrite guard
                for dc in range(8):
                    for nh in range(2):
                        mm = tensor.matmul(
                            out=ps_a[0:1, nh * 512:(nh + 1) * 512],
                            lhsT=w_col[b][:, dc:dc + 1],
                            rhs=wv_sb[:, dc, nh * 512:(nh + 1) * 512],
                            start=(dc == 0), stop=(dc == 7),
                        )
                mm.then_inc(sPE, 1)                      # PROJ{b}

            # batch-0 tail (fold0/proj0) threads through w1's pacing gaps
            w_chunk(0, 0)
            w_chunk(0, 1)
            qtb_mm(1)
            w_chunk(0, 2)
            w_chunk(0, 3)
            z_mm(0)
            w_chunk(1, 0)
            fold_mm(0)
            w_chunk(1, 1)
            proj_mm(0)
            w_chunk(1, 2)
            w_chunk(1, 3)
            z_mm(1)
            fold_mm(1)
            proj_mm(1)

        # ---------- DVE ----------
        @blk.vector
        def _(vector):
            vector.memset(ones_col[:], 1.0).then_inc(sDVE, 1)
            vector.memset(ones_bf[:], 1.0).then_inc(sDVE, 1)

            def small_chain(b, step):
                if step == 0:
                    if b == 0:
                        vector.wait_ge(sW, 64)
                    vector.wait_ge(sPE, PE[f"QPROW{b}"])
                    vector.tensor_add(qpr_sb[b][:], ps_small[0:1, 256:256 + A],
                                      bkq_row[:]).then_inc(sDVE, 1)   # QPRSB{b}
                elif step == 1:
                    vector.wait_ge(sPE, PE[f"QPF{b}"])
                    vector.tensor_copy(out=qp_sb[b][:], in_=ps_small[:, 0:2]) \
                        .then_inc(sDVE, 1)                            # QPSB{b}
                else:
                    vector.wait_ge(sPE, PE[f"QTB{b}"])
                    vector.tensor_copy(out=qtb_sb[b][:], in_=ps_b[:]) \
                        .then_inc(sDVE, 1)                            # QTBSB{b}

            def mult_chunk(b, c):
                g = b * NCHUNK + c
                vector.wait_ge(sK[g % KBUFS], (g // KBUFS + 1) * 16)
                if c == 0:
                    # self-wait: qtb_sb copy completion before reads
                    vector.wait_ge(sDVE, DVE[f"QTBSB{b}"])
                for j in range(CHUNK):
                    t = c * CHUNK + j
                    if j < NACT:
                        if g >= 1:
                            # scratch slot j: previous chunk's reduce done
                            bp, cp = divmod(g - 1, NCHUNK)
                            vector.wait_ge(sACT, ACT[red_ev(bp, cp, j)])
                        vector.tensor_mul(
                            scr[j][:], kt[g % KBUFS][:, j, :], qtb_sb[b][:]
                        ).then_inc(sDVE, 1)               # MUL{b}_{c}_{j}
                    else:
                        # fused dot product on DVE, in-place on the k slice:
                        # out=(k*1)*qt, accum_out=row sum
                        vector.scalar_tensor_tensor(
                            out=kt[g % KBUFS][:, j, :],
                            in0=kt[g % KBUFS][:, j, :], scalar=1.0,
                            in1=qtb_sb[b][:],
                            op0=AL.mult, op1=AL.mult,
                            accum_out=smat[b][:, t:t + 1],
                        ).then_inc(sDVE, 1)               # MUL{b}_{c}_{j}

            def zred(b):
                vector.wait_ge(sACT, ACT[f"EXPC{b}_{NCHUNK - 1}"])
                vector.reduce_sum(zredc[b][:], zpart[b][:],
                                  axis=mybir.AxisListType.X) \
                    .then_inc(sDVE, 1)                                # ZRED{b}

            def tail(b, step):
                if step == 0:
                    vector.wait_ge(sPE, PE[f"Z{b}"])
                    vector.reciprocal(invz[b][:], ps_small[0:1, 4:5]) \
                        .then_inc(sDVE, 1)                            # INVZ{b}
                elif step == 1:
                    vector.wait_ge(sPE, PE[f"FOLD{b}"])
                    vector.tensor_copy(out=w_col[b][:], in_=ps_small[:, 8:16]) \
                        .then_inc(sDVE, 1)                            # WCOL{b}
                else:
                    vector.wait_ge(sPE, PE[f"PROJ{b}"])
                    if b == 0:
                        vector.wait_ge(sBV, 16)
                    vector.tensor_add(o_sb[b][:], ps_a[0:1, :], bv_row[:]) \
                        .then_inc(sDVE, 1)                            # OSB{b}

            small_chain(0, 0)
            small_chain(0, 1)
            small_chain(0, 2)
            mult_chunk(0, 0)
            small_chain(1, 0)
            mult_chunk(0, 1)
            small_chain(1, 1)
            mult_chunk(0, 2)
            mult_chunk(0, 3)
            small_chain(1, 2)
            mult_chunk(1, 0)
            zred(0)
            mult_chunk(1, 1)
            tail(0, 0)          # INVZ0
            mult_chunk(1, 2)
            tail(0, 1)          # WCOL0
            mult_chunk(1, 3)
            tail(0, 2)          # OSB0
            zred(1)
            tail(1, 0)
            tail(1, 1)
            tail(1, 2)

        # ---------- ACT (scalar) ----------
        @blk.scalar
        def _(scalar):
            def qtsb(b):
                scalar.wait_ge(sPE, PE[f"QT{b}"])
                scalar.mul(qt_sb[b][:], ps_a[0:1, :], 1.0 / 16.0) \
                    .then_inc(sACT, 1)                                # QTSB{b}

            def red_chunk(b, c):
                for j in range(NACT):
                    t = c * CHUNK + j
                    scalar.wait_ge(sDVE, DVE[mult_ev(b, c, j)])
                    scalar.activation(
                        out=scr[j][:], in_=scr[j][:], func=AF.Copy,
                        accum_out=smat[b][:, t:t + 1],
                    ).then_inc(sACT, 1)                   # RED{b}_{c}_{j}

            def expc(b, c):
                # smat chunk writers: ACT reduces (self-order) + DVE stts
                scalar.wait_ge(sACT, ACT[red_ev(b, c, NACT - 1)])
                scalar.wait_ge(sDVE, DVE[mult_ev(b, c, CHUNK - 1)])
                scalar.activation(
                    out=pmat[b][:, c * CHUNK:(c + 1) * CHUNK],
                    in_=smat[b][:, c * CHUNK:(c + 1) * CHUNK], func=AF.Exp,
                    accum_out=zpart[b][:, c:c + 1],
                ).then_inc(sACT, 1)                                   # EXPC{b}_{c}

            def wrow(b):
                scalar.wait_ge(sPE, PE[f"W{b}C{NCHUNK - 1}"])
                scalar.wait_ge(sDVE, DVE[f"INVZ{b}"])
                acc = ps_w if b == 0 else ps_b
                scalar.activation(
                    out=w_row[b][:], in_=acc[0:1, :], func=AF.Copy,
                    bias=0.0, scale=invz[b][0:1, 0:1],
                ).then_inc(sACT, 1)                                   # WROW{b}

            qtsb(0)
            red_chunk(0, 0)
            expc(0, 0)
            red_chunk(0, 1)
            expc(0, 1)
            red_chunk(0, 2)
            expc(0, 2)
            qtsb(1)
            red_chunk(0, 3)
            expc(0, 3)
            red_chunk(1, 0)
            expc(1, 0)
            red_chunk(1, 1)
            expc(1, 1)
            wrow(0)
            red_chunk(1, 2)
            expc(1, 2)
            red_chunk(1, 3)
            expc(1, 3)
            wrow(1)

    return nc


_NC_CACHE = None


def get_nc():
    global _NC_CACHE
    if _NC_CACHE is None:
        _NC_CACHE = _build_nc()
    return _NC_CACHE


def make_in_maps(q, k, v, W_kq, b_kq, W_v, b_v):
    """Shard full inputs over 8 cores: batch-parallel, weights replicated.
    k, v, W_v are cast to bfloat16 on the host (compute dtype of the
    streaming contractions)."""
    import ml_dtypes

    bf16 = ml_dtypes.bfloat16
    # q pre-folded to column layout: qc[p, b, c] = q[b, c*128+p]
    q = np.asarray(q, dtype=np.float32).reshape(B, E)
    qc = np.ascontiguousarray(
        q.reshape(B, 8, 128).transpose(2, 0, 1).astype(bf16))  # [128, B, 8]
    k = np.ascontiguousarray(np.asarray(k, dtype=np.float32).astype(bf16))
    v = np.ascontiguousarray(np.asarray(v, dtype=np.float32).astype(bf16))
    W_kq32 = np.asarray(W_kq, dtype=np.float32)
    W_kq = np.ascontiguousarray(W_kq32.astype(bf16))
    W_kqT = np.ascontiguousarray(W_kq32.T.astype(bf16))
    b_kq = np.ascontiguousarray(np.asarray(b_kq, dtype=np.float32))
    W_v = np.ascontiguousarray(np.asarray(W_v, dtype=np.float32).astype(bf16))
    b_v = np.ascontiguousarray(np.asarray(b_v, dtype=np.float32))
    in_maps = []
    for i in range(NCORES):
        lo, hi = i * BPC, (i + 1) * BPC
        in_maps.append({
            "q": np.ascontiguousarray(qc[:, lo:hi, :]).reshape(128, BPC * 8),
            "k": k[lo:hi],
            "v": v[lo:hi],
            "W_kq": W_kq,
            "W_kqT": W_kqT,
            "b_kq": b_kq,
            "W_v": W_v,
            "b_v": b_v,
        })
    return in_maps


def kernel(q, k, v, W_kq, b_kq, W_v, b_v):
    from concourse.bass_utils import run_bass_kernel_spmd

    nc = get_nc()
    in_maps = make_in_maps(q, k, v, W_kq, b_kq, W_v, b_v)
    res = run_bass_kernel_spmd(nc, in_maps, core_ids=list(range(NCORES)))
    out = np.concatenate([res.results[i]["out"] for i in range(NCORES)], axis=0)
    return np.ascontiguousarray(out.astype(np.float32))
